# revision 21
# baseline (speedup 1.0000x reference)
"""CapsuleNet forward on 8 TRN2 NeuronCores, pure data-parallel over batch.

Per core (B=32): conv1(9x9 s1)+relu -> primary-caps conv(9x9 s2) -> squash ->
u_hat einsum (routing weights) -> 3 dynamic-routing iterations -> digit caps
-> classification norms + masked decoder MLP -> reconstruction.

Device layouts (see inline comments): conv stages keep channels on partitions
with free order (y, x, b); the routing phase keeps WU as [32j+b, g, cls, d]
where n = (8j + g//36)*36 + g%36 indexes the 1152 primary capsules.
"""
import sys
sys.path.insert(0, '/opt/trn_rl_repo')

import numpy as np
import ml_dtypes

import concourse.bass as bass
from concourse import bacc
import concourse.tile as tile
from concourse import mybir

f32 = mybir.dt.float32
bf16 = mybir.dt.bfloat16
npbf = ml_dtypes.bfloat16

B = 32            # per-core batch
NCLS = 10
DC = 16           # digit capsule dim
PC = 8            # primary capsule dim
NPRIM = 1152
CD = NCLS * DC    # 160

AF = mybir.ActivationFunctionType
ALU = mybir.AluOpType
AX = mybir.AxisListType

_CACHE = {}


def _build():
    nc = bacc.Bacc(None, target_bir_lowering=False)

    # ---- external inputs (per-core, host-prepped) ----
    d_imc = nc.dram_tensor("imc", [81, 400 * B], bf16, kind="ExternalInput")
    d_w1 = nc.dram_tensor("w1t", [81, 256], bf16, kind="ExternalInput")
    d_b1 = nc.dram_tensor("b1c", [128, 2], f32, kind="ExternalInput")
    d_pcw = nc.dram_tensor("pcwt", [81, 256, 256], bf16, kind="ExternalInput")
    d_pcb = nc.dram_tensor("pcbc", [128, 2], f32, kind="ExternalInput")
    d_rw = nc.dram_tensor("rw4", [4, 8, 288, 160], bf16, kind="ExternalInput")
    d_msk = nc.dram_tensor("mask32", [128, 32], bf16, kind="ExternalInput")
    d_ident = nc.dram_tensor("ident", [32, 32], bf16, kind="ExternalInput")
    d_oh = nc.dram_tensor("onehot", [32, 10], f32, kind="ExternalInput")
    d_w1d0 = nc.dram_tensor("w1d0", [128, 512], bf16, kind="ExternalInput")
    d_w1d1 = nc.dram_tensor("w1d1", [32, 512], bf16, kind="ExternalInput")
    d_w2d = nc.dram_tensor("w2d", [128, 4, 1024], bf16, kind="ExternalInput")
    d_w3d = nc.dram_tensor("w3d", [128, 8, 896], bf16, kind="ExternalInput")
    d_b1d = nc.dram_tensor("b1d", [128, 4], f32, kind="ExternalInput")
    d_b2d = nc.dram_tensor("b2d", [128, 8], f32, kind="ExternalInput")
    d_b3d = nc.dram_tensor("b3d", [128, 7], f32, kind="ExternalInput")

    # ---- outputs ----
    d_clf = nc.dram_tensor("clf_d", [32, 10], f32, kind="ExternalOutput")
    d_rec = nc.dram_tensor("recT_d", [128, 7, 32], f32, kind="ExternalOutput")

    # ---- internal scratch ----
    d_u = nc.dram_tensor("u_scr", [8, 4, 8, 1152], bf16, kind="Internal")

    with tile.TileContext(nc) as tc:
        _emit(nc, tc, locals())
    nc.compile()
    return nc


def _emit(nc, tc, d):
    import contextlib
    ctx = contextlib.ExitStack()
    with ctx:
        const = ctx.enter_context(tc.tile_pool(name="const", bufs=1))
        smallp = ctx.enter_context(tc.tile_pool(name="smallp", bufs=1))
        pcp = tc.alloc_tile_pool(name="pcp", bufs=1)
        xp = tc.alloc_tile_pool(name="xp", bufs=1)
        imcp = tc.alloc_tile_pool(name="imcp", bufs=1)
        psa_a = tc.alloc_tile_pool(name="psa_a", bufs=2, space="PSUM")

        # ============== constants to SBUF ==============
        czero = const.tile([128, 1], f32)
        nc.vector.memset(czero, 0.0)
        ceps = const.tile([128, 1], f32)
        nc.vector.memset(ceps, 1e-7)
        nc.const_aps.aps[(f32, 0.0)] = czero[:, :]
        nc.const_aps.aps[(f32, 1e-7)] = ceps[:, :]
        w1 = const.tile([81, 256], bf16)
        nc.sync.dma_start(out=w1, in_=d["d_w1"][:, :])
        b1 = const.tile([128, 2], f32)
        nc.sync.dma_start(out=b1, in_=d["d_b1"][:, :])
        pcb = const.tile([128, 2], f32)
        nc.sync.dma_start(out=pcb, in_=d["d_pcb"][:, :])
        msk = const.tile([128, 32], bf16)
        nc.sync.dma_start(out=msk, in_=d["d_msk"][:, :])

        # ============== stage A: conv1 + relu ==============
        # im2col [81, (y,x,b) 12800]; out x[c] [128, 12800] bf16, c = co-half
        imc = imcp.tile([81, 12800], bf16)
        for ic in range(5):
            nc.sync.dma_start(out=imc[:, 2560 * ic:2560 * (ic + 1)],
                              in_=d["d_imc"][:, 2560 * ic:2560 * (ic + 1)])
        x = [xp.tile([128, 12800], bf16, tag=f"x{c}", name=f"x{c}") for c in range(2)]
        for c in range(2):
            for t in range(25):
                pa = psa_a.tile([128, 512], f32)
                nc.tensor.matmul(pa[:, :], w1[:, 128 * c:128 * (c + 1)],
                                 imc[:, 512 * t:512 * (t + 1)],
                                 start=True, stop=True)
                # relu(x + b): alternate DVE / ACT
                o = x[c][:, 512 * t:512 * (t + 1)]
                if t % 2 == 0:
                    nc.scalar.activation(out=o, in_=pa[:, :], func=AF.Relu,
                                         bias=b1[:, c:c + 1], scale=1.0)
                else:
                    nc.vector.tensor_scalar(out=o, in0=pa[:, :],
                                            scalar1=b1[:, c:c + 1], scalar2=0.0,
                                            op0=ALU.add, op1=ALU.max)

        imcp.release()
        psa_a.release()

        # ============== stage B: primary caps conv (s2) ==============
        stream = tc.alloc_tile_pool(name="stream", bufs=2)
        psB = tc.alloc_tile_pool(name="psB", bufs=1, space="PSUM")
        # x[c] viewed (y20, x20, b32); out pc[c] [128, (yx36, b32)] f32
        pcs = [psB.tile([128, 384], f32, tag=f"pcs{i}", name=f"pcs{i}") for i in range(6)]
        TC = 9  # taps per stream chunk
        for tci in range(81 // TC):
            pw = stream.tile([128, TC, 2, 256], bf16, tag="pcw")
            for ci in range(2):
                nc.sync.dma_start(
                    out=pw[:, :, ci, :],
                    in_=d["d_pcw"][TC * tci:TC * (tci + 1),
                                   128 * ci:128 * (ci + 1), :]
                    .rearrange("t c o -> c t o"))
            for t in range(TC):
                tap = TC * tci + t
                ky, kx = divmod(tap, 9)
                for ci in range(2):
                    for co in range(2):
                        for nt in range(3):
                            rhs = bass.AP(
                                tensor=x[ci].tensor,
                                offset=x[ci].offset + (4 * nt + ky) * 640 + kx * 32,
                                ap=[x[ci].ap[0], [1280, 2], [64, 6], [1, 32]])
                            nc.tensor.matmul(
                                pcs[3 * co + nt][:, :],
                                pw[:, t, ci, 128 * co:128 * (co + 1)],
                                rhs,
                                start=(tap == 0 and ci == 0),
                                stop=(tap == 80 and ci == 1))
        pc = [pcp.tile([128, 1152], bf16, tag=f"pc{c}", name=f"pc{c}") for c in range(2)]
        with nc.allow_low_precision(reason="bf16 primary-caps activations"):
            for co in range(2):
                for nt in range(3):
                    nc.vector.tensor_scalar_add(
                        out=pc[co][:, 384 * nt:384 * (nt + 1)],
                        in0=pcs[3 * co + nt][:, :], scalar1=pcb[:, co:co + 1])

        stream.release()
        xp.release()

        # ============== squash -> u ==============
        sqp = tc.alloc_tile_pool(name="sqp", bufs=1)
        pcsq = [pcp.tile([128, 1152], bf16, tag=f"pcsq{c}", name=f"pcsq{c}") for c in range(2)]
        for c in range(2):
            nc.vector.tensor_tensor(out=pcsq[c], in0=pc[c], in1=pc[c], op=ALU.mult)
        psB.release()
        pssq = tc.alloc_tile_pool(name="pssq", bufs=1, space="PSUM")
        sps = [pssq.tile([32, 384], f32, tag=f"sps{i}", name=f"sps{i}") for i in range(3)]
        for nt in range(3):
            for c in range(2):
                nc.tensor.matmul(sps[nt][:, :], msk[:, :],
                                 pcsq[c][:, 384 * nt:384 * (nt + 1)],
                                 start=(c == 0), stop=(c == 1))
        t1 = sqp.tile([32, 1152], f32)
        t2 = sqp.tile([32, 1152], f32)
        for nt in range(3):
            sl = slice(384 * nt, 384 * (nt + 1))
            nc.scalar.activation(out=t1[:, sl], in_=sps[nt][:, :], func=AF.Sqrt,
                                 bias=1e-7, scale=1.0)
            nc.vector.tensor_scalar_add(out=t2[:, sl], in0=sps[nt][:, :],
                                        scalar1=1.0)
        r1 = sqp.tile([32, 1152], f32)
        nc.vector.reciprocal(out=r1, in_=t1)
        r2 = sqp.tile([32, 1152], f32)
        nc.vector.reciprocal(out=r2, in_=t2)
        fq = sqp.tile([32, 1152], f32)
        nc.vector.tensor_tensor(out=fq, in0=r1, in1=r2, op=ALU.mult)
        fqb = sqp.tile([32, 1152], bf16)
        for nt in range(3):
            sl = slice(384 * nt, 384 * (nt + 1))
            nc.vector.tensor_tensor(out=fqb[:, sl], in0=fq[:, sl],
                                    in1=sps[nt][:, :], op=ALU.mult)
        frep = sqp.tile([128, 1152], bf16)
        for q in range(4):
            nc.sync.dma_start(out=frep[32 * q:32 * (q + 1), :], in_=fqb[:, :])
        u = [sqp.tile([128, 1152], bf16, tag=f"u{c}", name=f"u{c}") for c in range(2)]
        for c in range(2):
            nc.vector.tensor_tensor(out=u[c], in0=pc[c], in1=frep, op=ALU.mult)

        # ============== u bounce through DRAM into strip layout ==============
        du = d["d_u"]
        for c in range(2):
            nc.sync.dma_start(
                out=du[4 * c:4 * (c + 1), :, :, :].rearrange("p j h f -> (p j h) f"),
                in_=u[c][:, :])
        sqp.release()
        pcp.release()
        pssq.release()
        wup = tc.alloc_tile_pool(name="wup", bufs=1)
        upadp = tc.alloc_tile_pool(name="upadp", bufs=1)
        rws = tc.alloc_tile_pool(name="rws", bufs=2)
        psE = tc.alloc_tile_pool(name="psE", bufs=2, space="PSUM")
        upad = upadp.tile([128, 288, 32], bf16)
        for j in range(4):
            nc.sync.dma_start(
                out=upad[32 * j:32 * j + 8, :, :]
                .rearrange("p (h y) b -> p h y b", h=8),
                in_=du[:, j, :, :].rearrange("p h (y b) -> p h y b", y=36))

        # ============== WU einsum (strip-parallel small-K matmuls) ==========
        wu = wup.tile([128, 288, 160], bf16)
        for gc in range(8):
            rwt = rws.tile([128, 36, 160], bf16, tag="rwt")
            for j in range(4):
                nc.sync.dma_start(out=rwt[32 * j:32 * j + 8, :, :],
                                  in_=d["d_rw"][j, :, 36 * gc:36 * (gc + 1), :])
            for gg in range(3):
                pe = psE.tile([128, 4, 512], f32, tag="pe")
                for gz in range(12):
                    gl = 12 * gg + gz
                    g = 36 * gc + gl
                    bk, g3 = divmod(gz, 3)
                    for j in range(4):
                        nc.tensor.matmul(
                            pe[32 * j:32 * (j + 1), bk, 160 * g3:160 * (g3 + 1)],
                            upad[32 * j:32 * j + 8, g, :],
                            rwt[32 * j:32 * j + 8, gl, :],
                            start=True, stop=True,
                            tile_position=(32 * j, 32 * j))
                o = wu[:, 36 * gc + 12 * gg:36 * gc + 12 * (gg + 1), :]
                o = o.rearrange("p a b -> p (a b)").rearrange(
                    "p (b x) -> p b x", b=4)
                i = bass.AP(tensor=pe.tensor, offset=pe.offset,
                            ap=[pe.ap[0], [512, 4], [1, 480]])
                if gg % 2 == 0:
                    nc.vector.tensor_copy(out=o, in_=i)
                else:
                    nc.scalar.copy(out=o, in_=i)

        rws.release()
        upadp.release()
        psE.release()

        # ============== routing ==============
        routp = tc.alloc_tile_pool(name="routp", bufs=1)
        dbuf = tc.alloc_tile_pool(name="dbuf", bufs=2)
        psR = tc.alloc_tile_pool(name="psR", bufs=2, space="PSUM")
        bij = routp.tile([128, 288, 10], f32)
        cbf = routp.tile([128, 288, 10], bf16)
        vrep = routp.tile([128, 160], bf16)
        tmpa = routp.tile([128, 48, 10], f32)
        tmpb = routp.tile([128, 48, 10], f32)
        v = smallp.tile([32, 160], f32)
        vpre = smallp.tile([32, 160], f32)
        vsb = smallp.tile([32, 160], bf16)
        vs = smallp.tile([32, 160], f32)
        sq = smallp.tile([32, 10], f32)
        w1q = smallp.tile([32, 10], f32)
        w2q = smallp.tile([32, 10], f32)
        fv = smallp.tile([32, 10], f32)
        exb = routp.tile([128, 288, 10], bf16)
        csum = routp.tile([128, 288], f32)
        crec = routp.tile([128, 288], bf16)



        def squash_v(scale):
            # reads vpre (sbuf); writes v (digit caps) and fv factors
            nc.scalar.activation(out=vs[:, :], in_=vpre[:, :], func=AF.Copy,
                                 bias=0.0, scale=scale)
            nc.vector.tensor_tensor(out=v[:, :], in0=vs, in1=vs, op=ALU.mult)
            vsqv = bass.AP(tensor=v.tensor, offset=v.offset,
                           ap=[v.ap[0], [1, 10], [10, 16]])
            nc.vector.tensor_reduce(out=sq[:, :], in_=vsqv,
                                    axis=AX.X, op=ALU.add)
            nc.scalar.activation(out=w1q[:, :], in_=sq[:, :], func=AF.Sqrt,
                                 bias=1e-7, scale=1.0)
            nc.vector.reciprocal(out=w1q, in_=w1q)
            nc.vector.tensor_scalar_add(out=w2q, in0=sq, scalar1=1.0)
            nc.vector.reciprocal(out=w2q, in_=w2q)
            nc.vector.tensor_tensor(out=fv, in0=w1q, in1=w2q, op=ALU.mult)
            nc.vector.tensor_tensor(out=fv, in0=fv, in1=sq, op=ALU.mult)
            fvb = bass.AP(tensor=fv.tensor, offset=fv.offset,
                          ap=[fv.ap[0], [0, 16], [1, 10]])
            nc.vector.tensor_tensor(
                out=v[:, :].rearrange("p (e c) -> p e c", e=16),
                in0=vs[:, :].rearrange("p (e c) -> p e c", e=16),
                in1=fvb, op=ALU.mult)

        def fold_squash(vpx, scale):
            fold = bass.AP(tensor=vpx.tensor, offset=vpx.offset,
                           ap=[vpx.ap[0], [1, 160], [160, 3]])
            nc.vector.tensor_reduce(out=vpre[:, :], in_=fold, axis=AX.X,
                                    op=ALU.add)
            squash_v(scale)

        def vrep_from_v(scale):
            nc.scalar.activation(out=vsb[:, :], in_=v[:, :], func=AF.Copy,
                                 bias=0.0, scale=scale)
            for q in range(4):
                nc.sync.dma_start(out=vrep[32 * q:32 * (q + 1), :],
                                  in_=vsb[:, :])

        def astep_chunk(gc, first):
            # b_ij[gc] (+)= c * sum_d WU*vrep  (vrep pre-scaled 0.1 when first)
            vb = bass.AP(tensor=vrep.tensor, offset=vrep.offset,
                         ap=[vrep.ap[0], [0, 48], [1, 160]])
            prod = dbuf.tile([128, 48, 160], bf16, tag="sprod", name="prod")
            nc.vector.tensor_tensor(
                out=prod[:, :, :], in0=wu[:, 48 * gc:48 * (gc + 1), :],
                in1=vb, op=ALU.mult)
            p4d = prod[:, :, :].rearrange("p g (e c) -> p g e c", e=16)
            t8 = dbuf.tile([128, 48, 8, 10], bf16, tag="tree", name="t8")
            nc.vector.tensor_tensor(out=t8, in0=p4d[:, :, 0:8, :],
                                    in1=p4d[:, :, 8:16, :], op=ALU.add)
            t4 = dbuf.tile([128, 48, 4, 10], bf16, tag="tree", name="t4")
            nc.vector.tensor_tensor(out=t4, in0=t8[:, :, 0:4, :],
                                    in1=t8[:, :, 4:8, :], op=ALU.add)
            t2 = dbuf.tile([128, 48, 2, 10], bf16, tag="tree", name="t2")
            nc.vector.tensor_tensor(out=t2, in0=t4[:, :, 0:2, :],
                                    in1=t4[:, :, 2:4, :], op=ALU.add)
            dst = (bij[:, 48 * gc:48 * (gc + 1), :] if first
                   else tmpa[:, :, :])
            nc.vector.tensor_tensor(out=dst, in0=t2[:, :, 0, :],
                                    in1=t2[:, :, 1, :], op=ALU.add)
            if not first:
                cb = cbf[:, 48 * gc:48 * (gc + 1), :]
                nc.vector.tensor_tensor(out=tmpb[:, :, :], in0=tmpa[:, :, :],
                                        in1=cb, op=ALU.mult)
                nc.vector.tensor_tensor(
                    out=bij[:, 48 * gc:48 * (gc + 1), :],
                    in0=bij[:, 48 * gc:48 * (gc + 1), :],
                    in1=tmpb[:, :, :], op=ALU.add)

        def softmax_chunk(gc):
            sl = slice(48 * gc, 48 * (gc + 1))
            nc.scalar.activation(
                out=exb[:, sl, :].rearrange("p a b -> p (a b)"),
                in_=bij[:, sl, :].rearrange("p a b -> p (a b)"),
                func=AF.Exp, bias=0.0, scale=1.0)
            nc.vector.tensor_reduce(out=csum[:, sl], in_=exb[:, sl, :],
                                    axis=AX.X, op=ALU.add)
            with nc.allow_low_precision(reason="bf16 softmax reciprocal"):
                nc.vector.reciprocal(out=crec[:, sl], in_=csum[:, sl])
            cr = bass.AP(tensor=crec.tensor, offset=crec.offset + 48 * gc,
                         ap=[crec.ap[0], [1, 48], [0, 10]])
            nc.vector.tensor_tensor(out=cbf[:, sl, :], in0=exb[:, sl, :],
                                    in1=cr, op=ALU.mult)

        def smult_mm_chunk(vpx, gc, use_c):
            if use_c:
                cb = cbf[:, 48 * gc:48 * (gc + 1), :]
                cb = bass.AP(tensor=cb.tensor, offset=cb.offset,
                             ap=[cb.ap[0], [10, 48], [0, 16], [1, 10]])
                sprod = dbuf.tile([128, 48, 160], bf16, tag="sprod",
                                  name="sprod")
                nc.vector.tensor_tensor(
                    out=sprod[:, :, :].rearrange("p g (e c) -> p g e c", e=16),
                    in0=wu[:, 48 * gc:48 * (gc + 1), :]
                    .rearrange("p g (e c) -> p g e c", e=16),
                    in1=cb, op=ALU.mult)
                srcs = sprod
            else:
                srcs = wu[:, 48 * gc:48 * (gc + 1), :]
            for tt in range(16):
                rhs = srcs[:, 3 * tt:3 * (tt + 1), :].rearrange(
                    "p a b -> p (a b)")
                nc.tensor.matmul(vpx[:, :], msk[:, :], rhs,
                                 start=(gc == 0 and tt == 0),
                                 stop=(gc == 5 and tt == 15))

        # ---- iter 0: v0 from raw WU ----
        vpx0 = psR.tile([32, 480], f32, tag="vpx", name="vpx0")
        for gc in range(6):
            smult_mm_chunk(vpx0, gc, use_c=False)
        fold_squash(vpx0, 0.1)
        vrep_from_v(0.1)
        # ---- astep0 + softmax1 + smult1 pipelined per chunk ----
        vpx1 = psR.tile([32, 480], f32, tag="vpx", name="vpx1")
        for gc in range(6):
            astep_chunk(gc, first=True)
            softmax_chunk(gc)
            smult_mm_chunk(vpx1, gc, use_c=True)
        fold_squash(vpx1, 1.0)
        vrep_from_v(1.0)
        # ---- astep1 + softmax2 + smult2 pipelined per chunk ----
        vpx2 = psR.tile([32, 480], f32, tag="vpx", name="vpx2")
        for gc in range(6):
            astep_chunk(gc, first=False)
            softmax_chunk(gc)
            smult_mm_chunk(vpx2, gc, use_c=True)
        fold_squash(vpx2, 1.0)

        # ============== clf output: ||v_final|| = sqrt(sq) * fv ==============
        clf = smallp.tile([32, 10], f32)
        nc.scalar.activation(out=clf, in_=sq[:, :], func=AF.Sqrt,
                             bias=0.0, scale=1.0)
        nc.vector.tensor_tensor(out=clf, in0=clf, in1=fv, op=ALU.mult)
        nc.sync.dma_start(out=d["d_clf"][:, :], in_=clf)

        dbuf.release()
        routp.release()
        wup.release()
        psR.release()

        # ============== decoder ==============
        decp = tc.alloc_tile_pool(name="decp", bufs=1)
        psD = tc.alloc_tile_pool(name="psD", bufs=2, space="PSUM")
        oh = decp.tile([32, 10], f32)
        nc.sync.dma_start(out=oh, in_=d["d_oh"][:, :])
        ident = decp.tile([32, 32], bf16)
        nc.sync.dma_start(out=ident, in_=d["d_ident"][:, :])
        wd10 = decp.tile([128, 512], bf16)
        nc.sync.dma_start(out=wd10, in_=d["d_w1d0"][:, :])
        wd11 = decp.tile([32, 512], bf16)
        nc.sync.dma_start(out=wd11, in_=d["d_w1d1"][:, :])
        wd2 = decp.tile([128, 4, 1024], bf16)
        nc.sync.dma_start(out=wd2, in_=d["d_w2d"][:, :, :])
        wd3 = decp.tile([128, 8, 896], bf16)
        nc.sync.dma_start(out=wd3, in_=d["d_w3d"][:, :, :])
        bd1 = decp.tile([128, 4], f32)
        nc.sync.dma_start(out=bd1, in_=d["d_b1d"][:, :])
        bd2 = decp.tile([128, 8], f32)
        nc.sync.dma_start(out=bd2, in_=d["d_b2d"][:, :])
        bd3 = decp.tile([128, 7], f32)
        nc.sync.dma_start(out=bd3, in_=d["d_b3d"][:, :])

        mskd = decp.tile([32, 160], bf16)
        ohb = bass.AP(tensor=oh.tensor, offset=oh.offset,
                      ap=[oh.ap[0], [0, 16], [1, 10]])
        nc.vector.tensor_tensor(
            out=mskd[:, :].rearrange("p (e c) -> p e c", e=16),
            in0=v[:, :].rearrange("p (e c) -> p e c", e=16),
            in1=ohb, op=ALU.mult)
        # transpose masked v -> h_T chunks [128, 32] + [32, 32]
        ph0 = psD.tile([128, 32], bf16, tag="ph0")
        nc.tensor.transpose(ph0[:, :], mskd[:, 0:128], ident[:, :])
        ph1 = psD.tile([32, 32], bf16, tag="ph1")
        nc.tensor.transpose(ph1[:, :], mskd[:, 128:160], ident[:, :])
        h0 = decp.tile([128, 32], bf16)
        nc.vector.tensor_copy(out=h0, in_=ph0[:, :])
        h1 = decp.tile([32, 32], bf16)
        nc.vector.tensor_copy(out=h1, in_=ph1[:, :])

        hd1 = decp.tile([128, 4, 32], bf16)
        for mt in range(4):
            pd = psD.tile([128, 32], f32, tag="pd")
            nc.tensor.matmul(pd[:, :], wd10[:, 128 * mt:128 * (mt + 1)],
                             h0[:, :], start=True, stop=False)
            nc.tensor.matmul(pd[:, :], wd11[:, 128 * mt:128 * (mt + 1)],
                             h1[:, :], start=False, stop=True)
            nc.scalar.activation(out=hd1[:, mt, :], in_=pd[:, :], func=AF.Relu,
                                 bias=bd1[:, mt:mt + 1], scale=1.0)
        hd2 = decp.tile([128, 8, 32], bf16)
        for mt in range(8):
            pd = psD.tile([128, 32], f32, tag="pd")
            for kc in range(4):
                nc.tensor.matmul(pd[:, :], wd2[:, kc, 128 * mt:128 * (mt + 1)],
                                 hd1[:, kc, :], start=(kc == 0), stop=(kc == 3))
            nc.scalar.activation(out=hd2[:, mt, :], in_=pd[:, :], func=AF.Relu,
                                 bias=bd2[:, mt:mt + 1], scale=1.0)
        rec = decp.tile([128, 7, 32], f32)
        for mt in range(7):
            pd = psD.tile([128, 32], f32, tag="pd")
            for kc in range(8):
                nc.tensor.matmul(pd[:, :], wd3[:, kc, 128 * mt:128 * (mt + 1)],
                                 hd2[:, kc, :], start=(kc == 0), stop=(kc == 7))
            nc.scalar.activation(out=rec[:, mt, :], in_=pd[:, :], func=AF.Sigmoid,
                                 bias=bd3[:, mt:mt + 1], scale=1.0)
        nc.sync.dma_start(out=d["d_rec"][:, :, :], in_=rec)
        decp.release()
        psD.release()


def _prep_shared(conv1_w, conv1_b, pc_w, pc_b, routing_weights,
                 dec_w1, dec_b1, dec_w2, dec_b2, dec_w3, dec_b3):
    s = {}
    s["w1t"] = np.ascontiguousarray(
        conv1_w.transpose(2, 3, 1, 0).reshape(81, 256)).astype(npbf)
    s["b1c"] = np.ascontiguousarray(conv1_b.reshape(2, 128).T).astype(np.float32)
    s["pcwt"] = np.ascontiguousarray(
        pc_w.transpose(2, 3, 1, 0).reshape(81, 256, 256)).astype(npbf)
    s["pcbc"] = np.ascontiguousarray(pc_b.reshape(2, 128).T).astype(np.float32)
    # rw4[j, p, g=(h,yx), cd]: RW[n, cls, d, p], n = (8j+h)*36+yx
    R = routing_weights.reshape(4, 8, 36, 10, 16, 8)  # [j, h, yx, cls, d, p]
    s["rw4"] = np.ascontiguousarray(
        R.transpose(0, 5, 1, 2, 4, 3).reshape(4, 8, 288, 160)).astype(npbf)
    s["mask32"] = ((np.arange(128)[:, None] % 32) ==
                   np.arange(32)[None, :]).astype(npbf)
    s["ident"] = np.eye(32).astype(npbf)
    w1r = dec_w1.reshape(10, 16, 512).transpose(1, 0, 2).reshape(160, 512)
    s["w1d0"] = np.ascontiguousarray(w1r[:128]).astype(npbf)
    s["w1d1"] = np.ascontiguousarray(w1r[128:]).astype(npbf)
    s["w2d"] = np.ascontiguousarray(
        dec_w2.reshape(4, 128, 1024).transpose(1, 0, 2)).astype(npbf)
    w3p = np.concatenate([dec_w3, np.zeros((1024, 112), dec_w3.dtype)], axis=1)
    s["w3d"] = np.ascontiguousarray(
        w3p.reshape(8, 128, 896).transpose(1, 0, 2)).astype(npbf)
    s["b1d"] = np.ascontiguousarray(dec_b1.reshape(4, 128).T).astype(np.float32)
    s["b2d"] = np.ascontiguousarray(dec_b2.reshape(8, 128).T).astype(np.float32)
    b3p = np.concatenate([dec_b3, np.zeros(112, dec_b3.dtype)])
    s["b3d"] = np.ascontiguousarray(b3p.reshape(7, 128).T).astype(np.float32)
    return s


def _prep_core(inputs_sh, labels_sh):
    m = {}
    arr = np.asarray(inputs_sh[:, 0], np.float32)          # [32, 28, 28]
    A = np.empty((9, 9, 20, 20, 32), np.float32)
    for ky in range(9):
        for kx in range(9):
            A[ky, kx] = arr[:, ky:ky + 20, kx:kx + 20].transpose(1, 2, 0)
    m["imc"] = A.reshape(81, 400 * 32).astype(npbf)
    oh = np.zeros((32, 10), np.float32)
    oh[np.arange(32), np.asarray(labels_sh)] = 1.0
    m["onehot"] = oh
    return m


def kernel(inputs, labels, conv1_w, conv1_b, pc_w, pc_b, routing_weights,
           dec_w1, dec_b1, dec_w2, dec_b2, dec_w3, dec_b3):
    from concourse.bass_utils import run_bass_kernel_spmd
    if "nc" not in _CACHE:
        _CACHE["nc"] = _build()
    nc = _CACHE["nc"]

    shared = _prep_shared(np.asarray(conv1_w, np.float32),
                          np.asarray(conv1_b, np.float32),
                          np.asarray(pc_w, np.float32),
                          np.asarray(pc_b, np.float32),
                          np.asarray(routing_weights, np.float32),
                          np.asarray(dec_w1, np.float32),
                          np.asarray(dec_b1, np.float32),
                          np.asarray(dec_w2, np.float32),
                          np.asarray(dec_b2, np.float32),
                          np.asarray(dec_w3, np.float32),
                          np.asarray(dec_b3, np.float32))
    in_maps = []
    for i in range(8):
        sh = slice(32 * i, 32 * (i + 1))
        m = dict(shared)
        m.update(_prep_core(np.asarray(inputs, np.float32)[sh],
                            np.asarray(labels)[sh]))
        in_maps.append(m)

    res = run_bass_kernel_spmd(nc, in_maps, core_ids=list(range(8)))

    clf = np.concatenate([res.results[i]["clf_d"] for i in range(8)], axis=0)
    recs = []
    for i in range(8):
        rt = res.results[i]["recT_d"]            # [128, 7, 32]
        r = rt.transpose(1, 0, 2).reshape(896, 32)[:784].T   # [32, 784]
        recs.append(r.reshape(32, 1, 28, 28))
    rec = np.concatenate(recs, axis=0)
    return clf.astype(np.float32), rec.astype(np.float32)


# revision 22
# speedup vs baseline: 1.0150x; 1.0150x over previous
"""CapsuleNet forward on 8 TRN2 NeuronCores, pure data-parallel over batch.

Per core (B=32): conv1(9x9 s1)+relu -> primary-caps conv(9x9 s2) -> squash ->
u_hat einsum (routing weights) -> 3 dynamic-routing iterations -> digit caps
-> classification norms + masked decoder MLP -> reconstruction.

Device layouts (see inline comments): conv stages keep channels on partitions
with free order (y, x, b); the routing phase keeps WU as [32j+b, g, cls, d]
where n = (8j + g//36)*36 + g%36 indexes the 1152 primary capsules.
"""
import sys
sys.path.insert(0, '/opt/trn_rl_repo')

import numpy as np
import ml_dtypes

import concourse.bass as bass
from concourse import bacc
import concourse.tile as tile
from concourse import mybir

f32 = mybir.dt.float32
bf16 = mybir.dt.bfloat16
npbf = ml_dtypes.bfloat16

B = 32            # per-core batch
NCLS = 10
DC = 16           # digit capsule dim
PC = 8            # primary capsule dim
NPRIM = 1152
CD = NCLS * DC    # 160

AF = mybir.ActivationFunctionType
ALU = mybir.AluOpType
AX = mybir.AxisListType

_CACHE = {}


def _build():
    nc = bacc.Bacc(None, target_bir_lowering=False)

    # ---- external inputs (per-core, host-prepped) ----
    d_imc = nc.dram_tensor("imc", [81, 400 * B], bf16, kind="ExternalInput")
    d_w1 = nc.dram_tensor("w1t", [81, 256], bf16, kind="ExternalInput")
    d_b1 = nc.dram_tensor("b1c", [128, 2], f32, kind="ExternalInput")
    d_pcw = nc.dram_tensor("pcwt", [81, 256, 256], bf16, kind="ExternalInput")
    d_pcb = nc.dram_tensor("pcbc", [128, 2], f32, kind="ExternalInput")
    d_rw = nc.dram_tensor("rw4", [4, 8, 288, 160], bf16, kind="ExternalInput")
    d_msk = nc.dram_tensor("mask32", [128, 32], bf16, kind="ExternalInput")
    d_ident = nc.dram_tensor("ident", [32, 32], bf16, kind="ExternalInput")
    d_oh = nc.dram_tensor("onehot", [32, 10], f32, kind="ExternalInput")
    d_w1d0 = nc.dram_tensor("w1d0", [128, 512], bf16, kind="ExternalInput")
    d_w1d1 = nc.dram_tensor("w1d1", [32, 512], bf16, kind="ExternalInput")
    d_w2d = nc.dram_tensor("w2d", [128, 4, 1024], bf16, kind="ExternalInput")
    d_w3d = nc.dram_tensor("w3d", [128, 8, 896], bf16, kind="ExternalInput")
    d_b1d = nc.dram_tensor("b1d", [128, 4], f32, kind="ExternalInput")
    d_b2d = nc.dram_tensor("b2d", [128, 8], f32, kind="ExternalInput")
    d_b3d = nc.dram_tensor("b3d", [128, 7], f32, kind="ExternalInput")

    # ---- outputs ----
    d_clf = nc.dram_tensor("clf_d", [32, 10], f32, kind="ExternalOutput")
    d_rec = nc.dram_tensor("recT_d", [128, 7, 32], f32, kind="ExternalOutput")

    # ---- internal scratch ----
    d_u = nc.dram_tensor("u_scr", [8, 4, 8, 1152], bf16, kind="Internal")

    with tile.TileContext(nc) as tc:
        _emit(nc, tc, locals())
    nc.compile()
    return nc


def _emit(nc, tc, d):
    import contextlib
    ctx = contextlib.ExitStack()
    with ctx:
        const = ctx.enter_context(tc.tile_pool(name="const", bufs=1))
        smallp = ctx.enter_context(tc.tile_pool(name="smallp", bufs=1))
        pcp = tc.alloc_tile_pool(name="pcp", bufs=1)
        xp = tc.alloc_tile_pool(name="xp", bufs=1)
        imcp = tc.alloc_tile_pool(name="imcp", bufs=1)
        psa_a = tc.alloc_tile_pool(name="psa_a", bufs=2, space="PSUM")

        # ============== constants to SBUF ==============
        czero = const.tile([128, 1], f32)
        nc.vector.memset(czero, 0.0)
        ceps = const.tile([128, 1], f32)
        nc.vector.memset(ceps, 1e-7)
        nc.const_aps.aps[(f32, 0.0)] = czero[:, :]
        nc.const_aps.aps[(f32, 1e-7)] = ceps[:, :]
        w1 = const.tile([81, 256], bf16)
        nc.sync.dma_start(out=w1, in_=d["d_w1"][:, :])
        b1 = const.tile([128, 2], f32)
        nc.sync.dma_start(out=b1, in_=d["d_b1"][:, :])
        pcb = const.tile([128, 2], f32)
        nc.sync.dma_start(out=pcb, in_=d["d_pcb"][:, :])
        msk = const.tile([128, 32], bf16)
        nc.sync.dma_start(out=msk, in_=d["d_msk"][:, :])

        # ============== stage A: conv1 + relu ==============
        # im2col [81, (y,x,b) 12800]; out x[c] [128, 12800] bf16, c = co-half
        imc = imcp.tile([81, 12800], bf16)
        for ic in range(5):
            nc.sync.dma_start(out=imc[:, 2560 * ic:2560 * (ic + 1)],
                              in_=d["d_imc"][:, 2560 * ic:2560 * (ic + 1)])
        x = [xp.tile([128, 12800], bf16, tag=f"x{c}", name=f"x{c}") for c in range(2)]
        for c in range(2):
            for t in range(25):
                pa = psa_a.tile([128, 512], f32)
                nc.tensor.matmul(pa[:, :], w1[:, 128 * c:128 * (c + 1)],
                                 imc[:, 512 * t:512 * (t + 1)],
                                 start=True, stop=True)
                # relu(x + b): alternate DVE / ACT
                o = x[c][:, 512 * t:512 * (t + 1)]
                if t % 2 == 0:
                    nc.scalar.activation(out=o, in_=pa[:, :], func=AF.Relu,
                                         bias=b1[:, c:c + 1], scale=1.0)
                else:
                    nc.vector.tensor_scalar(out=o, in0=pa[:, :],
                                            scalar1=b1[:, c:c + 1], scalar2=0.0,
                                            op0=ALU.add, op1=ALU.max)

        imcp.release()
        psa_a.release()

        # ============== stage B: primary caps conv (s2) ==============
        stream = tc.alloc_tile_pool(name="stream", bufs=2)
        psB = tc.alloc_tile_pool(name="psB", bufs=1, space="PSUM")
        # x[c] viewed (y20, x20, b32); out pc[c] [128, (yx36, b32)] f32
        pcs = [psB.tile([128, 384], f32, tag=f"pcs{i}", name=f"pcs{i}") for i in range(6)]
        TC = 9  # taps per stream chunk
        for tci in range(81 // TC):
            pw = stream.tile([128, TC, 2, 256], bf16, tag="pcw")
            for ci in range(2):
                nc.sync.dma_start(
                    out=pw[:, :, ci, :],
                    in_=d["d_pcw"][TC * tci:TC * (tci + 1),
                                   128 * ci:128 * (ci + 1), :]
                    .rearrange("t c o -> c t o"))
            for t in range(TC):
                tap = TC * tci + t
                ky, kx = divmod(tap, 9)
                for ci in range(2):
                    for co in range(2):
                        for nt in range(3):
                            rhs = bass.AP(
                                tensor=x[ci].tensor,
                                offset=x[ci].offset + (4 * nt + ky) * 640 + kx * 32,
                                ap=[x[ci].ap[0], [1280, 2], [64, 6], [1, 32]])
                            nc.tensor.matmul(
                                pcs[3 * co + nt][:, :],
                                pw[:, t, ci, 128 * co:128 * (co + 1)],
                                rhs,
                                start=(tap == 0 and ci == 0),
                                stop=(tap == 80 and ci == 1))
        pc = [pcp.tile([128, 1152], bf16, tag=f"pc{c}", name=f"pc{c}") for c in range(2)]
        with nc.allow_low_precision(reason="bf16 primary-caps activations"):
            for co in range(2):
                for nt in range(3):
                    nc.vector.tensor_scalar_add(
                        out=pc[co][:, 384 * nt:384 * (nt + 1)],
                        in0=pcs[3 * co + nt][:, :], scalar1=pcb[:, co:co + 1])

        stream.release()
        xp.release()

        # ============== squash -> u ==============
        sqp = tc.alloc_tile_pool(name="sqp", bufs=1)
        pcsq = [pcp.tile([128, 1152], bf16, tag=f"pcsq{c}", name=f"pcsq{c}") for c in range(2)]
        for c in range(2):
            nc.vector.tensor_tensor(out=pcsq[c], in0=pc[c], in1=pc[c], op=ALU.mult)
        psB.release()
        pssq = tc.alloc_tile_pool(name="pssq", bufs=1, space="PSUM")
        sps = [pssq.tile([32, 384], f32, tag=f"sps{i}", name=f"sps{i}") for i in range(3)]
        for nt in range(3):
            for c in range(2):
                nc.tensor.matmul(sps[nt][:, :], msk[:, :],
                                 pcsq[c][:, 384 * nt:384 * (nt + 1)],
                                 start=(c == 0), stop=(c == 1))
        t1 = sqp.tile([32, 1152], f32)
        t2 = sqp.tile([32, 1152], f32)
        for nt in range(3):
            sl = slice(384 * nt, 384 * (nt + 1))
            nc.scalar.activation(out=t1[:, sl], in_=sps[nt][:, :], func=AF.Sqrt,
                                 bias=1e-7, scale=1.0)
            nc.vector.tensor_scalar_add(out=t2[:, sl], in0=sps[nt][:, :],
                                        scalar1=1.0)
        r1 = sqp.tile([32, 1152], f32)
        nc.vector.reciprocal(out=r1, in_=t1)
        r2 = sqp.tile([32, 1152], f32)
        nc.vector.reciprocal(out=r2, in_=t2)
        fq = sqp.tile([32, 1152], f32)
        nc.vector.tensor_tensor(out=fq, in0=r1, in1=r2, op=ALU.mult)
        fqb = sqp.tile([32, 1152], bf16)
        for nt in range(3):
            sl = slice(384 * nt, 384 * (nt + 1))
            nc.vector.tensor_tensor(out=fqb[:, sl], in0=fq[:, sl],
                                    in1=sps[nt][:, :], op=ALU.mult)
        frep = sqp.tile([128, 1152], bf16)
        for q in range(4):
            nc.sync.dma_start(out=frep[32 * q:32 * (q + 1), :], in_=fqb[:, :])
        u = [sqp.tile([128, 1152], bf16, tag=f"u{c}", name=f"u{c}") for c in range(2)]
        for c in range(2):
            nc.vector.tensor_tensor(out=u[c], in0=pc[c], in1=frep, op=ALU.mult)

        # ============== u bounce through DRAM into strip layout ==============
        du = d["d_u"]
        for c in range(2):
            nc.sync.dma_start(
                out=du[4 * c:4 * (c + 1), :, :, :].rearrange("p j h f -> (p j h) f"),
                in_=u[c][:, :])
        sqp.release()
        pcp.release()
        pssq.release()
        wup = tc.alloc_tile_pool(name="wup", bufs=1)
        upadp = tc.alloc_tile_pool(name="upadp", bufs=1)
        rws = tc.alloc_tile_pool(name="rws", bufs=2)
        psE = tc.alloc_tile_pool(name="psE", bufs=3, space="PSUM")
        upad = upadp.tile([128, 288, 32], bf16)
        for j in range(4):
            nc.sync.dma_start(
                out=upad[32 * j:32 * j + 8, :, :]
                .rearrange("p (h y) b -> p h y b", h=8),
                in_=du[:, j, :, :].rearrange("p h (y b) -> p h y b", y=36))

        # ============== WU einsum (strip-parallel small-K matmuls) ==========
        wu = wup.tile([128, 288, 160], bf16)
        for gc in range(8):
            rwt = rws.tile([128, 36, 160], bf16, tag="rwt")
            for j in range(4):
                nc.sync.dma_start(out=rwt[32 * j:32 * j + 8, :, :],
                                  in_=d["d_rw"][j, :, 36 * gc:36 * (gc + 1), :])
            for gg in range(6):
                pe = psE.tile([128, 2, 512], f32, tag="pe")
                for g6 in range(6):
                    gl = 6 * gg + g6
                    g = 36 * gc + gl
                    bk, g3 = divmod(g6, 3)
                    for j in range(4):
                        nc.tensor.matmul(
                            pe[32 * j:32 * (j + 1), bk, 160 * g3:160 * (g3 + 1)],
                            upad[32 * j:32 * j + 8, g, :],
                            rwt[32 * j:32 * j + 8, gl, :],
                            start=True, stop=True,
                            tile_position=(32 * j, 32 * j))
                o = wu[:, 36 * gc + 6 * gg:36 * gc + 6 * (gg + 1), :]
                o = o.rearrange("p a b -> p (a b)").rearrange(
                    "p (b x) -> p b x", b=2)
                i = bass.AP(tensor=pe.tensor, offset=pe.offset,
                            ap=[pe.ap[0], [512, 2], [1, 480]])
                if gg % 2 == 0:
                    nc.vector.tensor_copy(out=o, in_=i)
                else:
                    nc.scalar.copy(out=o, in_=i)

        rws.release()
        upadp.release()
        psE.release()

        # ============== routing ==============
        routp = tc.alloc_tile_pool(name="routp", bufs=1)
        dbuf = tc.alloc_tile_pool(name="dbuf", bufs=2)
        psR = tc.alloc_tile_pool(name="psR", bufs=2, space="PSUM")
        bij = routp.tile([128, 288, 10], f32)
        cbf = routp.tile([128, 288, 10], bf16)
        vrep = routp.tile([128, 160], bf16)
        tmpa = routp.tile([128, 48, 10], f32)
        tmpb = routp.tile([128, 48, 10], f32)
        v = smallp.tile([32, 160], f32)
        vpre = smallp.tile([32, 160], f32)
        vsb = smallp.tile([32, 160], bf16)
        vs = smallp.tile([32, 160], f32)
        sq = smallp.tile([32, 10], f32)
        w1q = smallp.tile([32, 10], f32)
        w2q = smallp.tile([32, 10], f32)
        fv = smallp.tile([32, 10], f32)
        exb = routp.tile([128, 288, 10], bf16)
        csum = routp.tile([128, 288], f32)
        crec = routp.tile([128, 288], bf16)



        def squash_v(scale):
            # reads vpre (sbuf); writes v (digit caps) and fv factors
            nc.scalar.activation(out=vs[:, :], in_=vpre[:, :], func=AF.Copy,
                                 bias=0.0, scale=scale)
            nc.vector.tensor_tensor(out=v[:, :], in0=vs, in1=vs, op=ALU.mult)
            vsqv = bass.AP(tensor=v.tensor, offset=v.offset,
                           ap=[v.ap[0], [1, 10], [10, 16]])
            nc.vector.tensor_reduce(out=sq[:, :], in_=vsqv,
                                    axis=AX.X, op=ALU.add)
            nc.scalar.activation(out=w1q[:, :], in_=sq[:, :], func=AF.Sqrt,
                                 bias=1e-7, scale=1.0)
            nc.vector.reciprocal(out=w1q, in_=w1q)
            nc.vector.tensor_scalar_add(out=w2q, in0=sq, scalar1=1.0)
            nc.vector.reciprocal(out=w2q, in_=w2q)
            nc.vector.tensor_tensor(out=fv, in0=w1q, in1=w2q, op=ALU.mult)
            nc.vector.tensor_tensor(out=fv, in0=fv, in1=sq, op=ALU.mult)
            fvb = bass.AP(tensor=fv.tensor, offset=fv.offset,
                          ap=[fv.ap[0], [0, 16], [1, 10]])
            nc.vector.tensor_tensor(
                out=v[:, :].rearrange("p (e c) -> p e c", e=16),
                in0=vs[:, :].rearrange("p (e c) -> p e c", e=16),
                in1=fvb, op=ALU.mult)

        def fold_squash(vpx, scale):
            fold = bass.AP(tensor=vpx.tensor, offset=vpx.offset,
                           ap=[vpx.ap[0], [1, 160], [160, 3]])
            nc.vector.tensor_reduce(out=vpre[:, :], in_=fold, axis=AX.X,
                                    op=ALU.add)
            squash_v(scale)

        def vrep_from_v(scale):
            nc.scalar.activation(out=vsb[:, :], in_=v[:, :], func=AF.Copy,
                                 bias=0.0, scale=scale)
            for q in range(4):
                nc.sync.dma_start(out=vrep[32 * q:32 * (q + 1), :],
                                  in_=vsb[:, :])

        def astep_chunk(gc, first):
            # b_ij[gc] (+)= c * sum_d WU*vrep  (vrep pre-scaled 0.1 when first)
            vb = bass.AP(tensor=vrep.tensor, offset=vrep.offset,
                         ap=[vrep.ap[0], [0, 48], [1, 160]])
            prod = dbuf.tile([128, 48, 160], bf16, tag="sprod", name="prod")
            nc.vector.tensor_tensor(
                out=prod[:, :, :], in0=wu[:, 48 * gc:48 * (gc + 1), :],
                in1=vb, op=ALU.mult)
            p4d = prod[:, :, :].rearrange("p g (e c) -> p g e c", e=16)
            t8 = dbuf.tile([128, 48, 8, 10], bf16, tag="tree", name="t8")
            nc.vector.tensor_tensor(out=t8, in0=p4d[:, :, 0:8, :],
                                    in1=p4d[:, :, 8:16, :], op=ALU.add)
            t4 = dbuf.tile([128, 48, 4, 10], bf16, tag="tree", name="t4")
            nc.vector.tensor_tensor(out=t4, in0=t8[:, :, 0:4, :],
                                    in1=t8[:, :, 4:8, :], op=ALU.add)
            t2 = dbuf.tile([128, 48, 2, 10], bf16, tag="tree", name="t2")
            nc.vector.tensor_tensor(out=t2, in0=t4[:, :, 0:2, :],
                                    in1=t4[:, :, 2:4, :], op=ALU.add)
            dst = (bij[:, 48 * gc:48 * (gc + 1), :] if first
                   else tmpa[:, :, :])
            nc.vector.tensor_tensor(out=dst, in0=t2[:, :, 0, :],
                                    in1=t2[:, :, 1, :], op=ALU.add)
            if not first:
                cb = cbf[:, 48 * gc:48 * (gc + 1), :]
                nc.vector.tensor_tensor(out=tmpb[:, :, :], in0=tmpa[:, :, :],
                                        in1=cb, op=ALU.mult)
                nc.vector.tensor_tensor(
                    out=bij[:, 48 * gc:48 * (gc + 1), :],
                    in0=bij[:, 48 * gc:48 * (gc + 1), :],
                    in1=tmpb[:, :, :], op=ALU.add)

        def softmax_chunk(gc):
            sl = slice(48 * gc, 48 * (gc + 1))
            nc.scalar.activation(
                out=exb[:, sl, :].rearrange("p a b -> p (a b)"),
                in_=bij[:, sl, :].rearrange("p a b -> p (a b)"),
                func=AF.Exp, bias=0.0, scale=1.0)
            nc.vector.tensor_reduce(out=csum[:, sl], in_=exb[:, sl, :],
                                    axis=AX.X, op=ALU.add)
            with nc.allow_low_precision(reason="bf16 softmax reciprocal"):
                nc.vector.reciprocal(out=crec[:, sl], in_=csum[:, sl])
            cr = bass.AP(tensor=crec.tensor, offset=crec.offset + 48 * gc,
                         ap=[crec.ap[0], [1, 48], [0, 10]])
            nc.vector.tensor_tensor(out=cbf[:, sl, :], in0=exb[:, sl, :],
                                    in1=cr, op=ALU.mult)

        def smult_mm_chunk(vpx, gc, use_c):
            if use_c:
                cb = cbf[:, 48 * gc:48 * (gc + 1), :]
                cb = bass.AP(tensor=cb.tensor, offset=cb.offset,
                             ap=[cb.ap[0], [10, 48], [0, 16], [1, 10]])
                sprod = dbuf.tile([128, 48, 160], bf16, tag="sprod",
                                  name="sprod")
                nc.vector.tensor_tensor(
                    out=sprod[:, :, :].rearrange("p g (e c) -> p g e c", e=16),
                    in0=wu[:, 48 * gc:48 * (gc + 1), :]
                    .rearrange("p g (e c) -> p g e c", e=16),
                    in1=cb, op=ALU.mult)
                srcs = sprod
            else:
                srcs = wu[:, 48 * gc:48 * (gc + 1), :]
            for tt in range(16):
                rhs = srcs[:, 3 * tt:3 * (tt + 1), :].rearrange(
                    "p a b -> p (a b)")
                nc.tensor.matmul(vpx[:, :], msk[:, :], rhs,
                                 start=(gc == 0 and tt == 0),
                                 stop=(gc == 5 and tt == 15))

        # ---- iter 0: v0 from raw WU ----
        vpx0 = psR.tile([32, 480], f32, tag="vpx", name="vpx0")
        for gc in range(6):
            smult_mm_chunk(vpx0, gc, use_c=False)
        fold_squash(vpx0, 0.1)
        vrep_from_v(0.1)
        # ---- astep0 + softmax1 + smult1 pipelined per chunk ----
        vpx1 = psR.tile([32, 480], f32, tag="vpx", name="vpx1")
        for gc in range(6):
            astep_chunk(gc, first=True)
            softmax_chunk(gc)
            smult_mm_chunk(vpx1, gc, use_c=True)
        fold_squash(vpx1, 1.0)
        vrep_from_v(1.0)
        # ---- astep1 + softmax2 + smult2 pipelined per chunk ----
        vpx2 = psR.tile([32, 480], f32, tag="vpx", name="vpx2")
        for gc in range(6):
            astep_chunk(gc, first=False)
            softmax_chunk(gc)
            smult_mm_chunk(vpx2, gc, use_c=True)
        fold_squash(vpx2, 1.0)

        # ============== clf output: ||v_final|| = sqrt(sq) * fv ==============
        clf = smallp.tile([32, 10], f32)
        nc.scalar.activation(out=clf, in_=sq[:, :], func=AF.Sqrt,
                             bias=0.0, scale=1.0)
        nc.vector.tensor_tensor(out=clf, in0=clf, in1=fv, op=ALU.mult)
        nc.sync.dma_start(out=d["d_clf"][:, :], in_=clf)

        dbuf.release()
        routp.release()
        wup.release()
        psR.release()

        # ============== decoder ==============
        decp = tc.alloc_tile_pool(name="decp", bufs=1)
        psD = tc.alloc_tile_pool(name="psD", bufs=2, space="PSUM")
        oh = decp.tile([32, 10], f32)
        nc.sync.dma_start(out=oh, in_=d["d_oh"][:, :])
        ident = decp.tile([32, 32], bf16)
        nc.sync.dma_start(out=ident, in_=d["d_ident"][:, :])
        wd10 = decp.tile([128, 512], bf16)
        nc.sync.dma_start(out=wd10, in_=d["d_w1d0"][:, :])
        wd11 = decp.tile([32, 512], bf16)
        nc.sync.dma_start(out=wd11, in_=d["d_w1d1"][:, :])
        wd2 = decp.tile([128, 4, 1024], bf16)
        nc.sync.dma_start(out=wd2, in_=d["d_w2d"][:, :, :])
        wd3 = decp.tile([128, 8, 896], bf16)
        nc.sync.dma_start(out=wd3, in_=d["d_w3d"][:, :, :])
        bd1 = decp.tile([128, 4], f32)
        nc.sync.dma_start(out=bd1, in_=d["d_b1d"][:, :])
        bd2 = decp.tile([128, 8], f32)
        nc.sync.dma_start(out=bd2, in_=d["d_b2d"][:, :])
        bd3 = decp.tile([128, 7], f32)
        nc.sync.dma_start(out=bd3, in_=d["d_b3d"][:, :])

        mskd = decp.tile([32, 160], bf16)
        ohb = bass.AP(tensor=oh.tensor, offset=oh.offset,
                      ap=[oh.ap[0], [0, 16], [1, 10]])
        nc.vector.tensor_tensor(
            out=mskd[:, :].rearrange("p (e c) -> p e c", e=16),
            in0=v[:, :].rearrange("p (e c) -> p e c", e=16),
            in1=ohb, op=ALU.mult)
        # transpose masked v -> h_T chunks [128, 32] + [32, 32]
        ph0 = psD.tile([128, 32], bf16, tag="ph0")
        nc.tensor.transpose(ph0[:, :], mskd[:, 0:128], ident[:, :])
        ph1 = psD.tile([32, 32], bf16, tag="ph1")
        nc.tensor.transpose(ph1[:, :], mskd[:, 128:160], ident[:, :])
        h0 = decp.tile([128, 32], bf16)
        nc.vector.tensor_copy(out=h0, in_=ph0[:, :])
        h1 = decp.tile([32, 32], bf16)
        nc.vector.tensor_copy(out=h1, in_=ph1[:, :])

        hd1 = decp.tile([128, 4, 32], bf16)
        for mt in range(4):
            pd = psD.tile([128, 32], f32, tag="pd")
            nc.tensor.matmul(pd[:, :], wd10[:, 128 * mt:128 * (mt + 1)],
                             h0[:, :], start=True, stop=False)
            nc.tensor.matmul(pd[:, :], wd11[:, 128 * mt:128 * (mt + 1)],
                             h1[:, :], start=False, stop=True)
            nc.scalar.activation(out=hd1[:, mt, :], in_=pd[:, :], func=AF.Relu,
                                 bias=bd1[:, mt:mt + 1], scale=1.0)
        hd2 = decp.tile([128, 8, 32], bf16)
        for mt in range(8):
            pd = psD.tile([128, 32], f32, tag="pd")
            for kc in range(4):
                nc.tensor.matmul(pd[:, :], wd2[:, kc, 128 * mt:128 * (mt + 1)],
                                 hd1[:, kc, :], start=(kc == 0), stop=(kc == 3))
            nc.scalar.activation(out=hd2[:, mt, :], in_=pd[:, :], func=AF.Relu,
                                 bias=bd2[:, mt:mt + 1], scale=1.0)
        rec = decp.tile([128, 7, 32], f32)
        for mt in range(7):
            pd = psD.tile([128, 32], f32, tag="pd")
            for kc in range(8):
                nc.tensor.matmul(pd[:, :], wd3[:, kc, 128 * mt:128 * (mt + 1)],
                                 hd2[:, kc, :], start=(kc == 0), stop=(kc == 7))
            nc.scalar.activation(out=rec[:, mt, :], in_=pd[:, :], func=AF.Sigmoid,
                                 bias=bd3[:, mt:mt + 1], scale=1.0)
        nc.sync.dma_start(out=d["d_rec"][:, :, :], in_=rec)
        decp.release()
        psD.release()


def _prep_shared(conv1_w, conv1_b, pc_w, pc_b, routing_weights,
                 dec_w1, dec_b1, dec_w2, dec_b2, dec_w3, dec_b3):
    s = {}
    s["w1t"] = np.ascontiguousarray(
        conv1_w.transpose(2, 3, 1, 0).reshape(81, 256)).astype(npbf)
    s["b1c"] = np.ascontiguousarray(conv1_b.reshape(2, 128).T).astype(np.float32)
    s["pcwt"] = np.ascontiguousarray(
        pc_w.transpose(2, 3, 1, 0).reshape(81, 256, 256)).astype(npbf)
    s["pcbc"] = np.ascontiguousarray(pc_b.reshape(2, 128).T).astype(np.float32)
    # rw4[j, p, g=(h,yx), cd]: RW[n, cls, d, p], n = (8j+h)*36+yx
    R = routing_weights.reshape(4, 8, 36, 10, 16, 8)  # [j, h, yx, cls, d, p]
    s["rw4"] = np.ascontiguousarray(
        R.transpose(0, 5, 1, 2, 4, 3).reshape(4, 8, 288, 160)).astype(npbf)
    s["mask32"] = ((np.arange(128)[:, None] % 32) ==
                   np.arange(32)[None, :]).astype(npbf)
    s["ident"] = np.eye(32).astype(npbf)
    w1r = dec_w1.reshape(10, 16, 512).transpose(1, 0, 2).reshape(160, 512)
    s["w1d0"] = np.ascontiguousarray(w1r[:128]).astype(npbf)
    s["w1d1"] = np.ascontiguousarray(w1r[128:]).astype(npbf)
    s["w2d"] = np.ascontiguousarray(
        dec_w2.reshape(4, 128, 1024).transpose(1, 0, 2)).astype(npbf)
    w3p = np.concatenate([dec_w3, np.zeros((1024, 112), dec_w3.dtype)], axis=1)
    s["w3d"] = np.ascontiguousarray(
        w3p.reshape(8, 128, 896).transpose(1, 0, 2)).astype(npbf)
    s["b1d"] = np.ascontiguousarray(dec_b1.reshape(4, 128).T).astype(np.float32)
    s["b2d"] = np.ascontiguousarray(dec_b2.reshape(8, 128).T).astype(np.float32)
    b3p = np.concatenate([dec_b3, np.zeros(112, dec_b3.dtype)])
    s["b3d"] = np.ascontiguousarray(b3p.reshape(7, 128).T).astype(np.float32)
    return s


def _prep_core(inputs_sh, labels_sh):
    m = {}
    arr = np.asarray(inputs_sh[:, 0], np.float32)          # [32, 28, 28]
    A = np.empty((9, 9, 20, 20, 32), np.float32)
    for ky in range(9):
        for kx in range(9):
            A[ky, kx] = arr[:, ky:ky + 20, kx:kx + 20].transpose(1, 2, 0)
    m["imc"] = A.reshape(81, 400 * 32).astype(npbf)
    oh = np.zeros((32, 10), np.float32)
    oh[np.arange(32), np.asarray(labels_sh)] = 1.0
    m["onehot"] = oh
    return m


def kernel(inputs, labels, conv1_w, conv1_b, pc_w, pc_b, routing_weights,
           dec_w1, dec_b1, dec_w2, dec_b2, dec_w3, dec_b3):
    from concourse.bass_utils import run_bass_kernel_spmd
    if "nc" not in _CACHE:
        _CACHE["nc"] = _build()
    nc = _CACHE["nc"]

    shared = _prep_shared(np.asarray(conv1_w, np.float32),
                          np.asarray(conv1_b, np.float32),
                          np.asarray(pc_w, np.float32),
                          np.asarray(pc_b, np.float32),
                          np.asarray(routing_weights, np.float32),
                          np.asarray(dec_w1, np.float32),
                          np.asarray(dec_b1, np.float32),
                          np.asarray(dec_w2, np.float32),
                          np.asarray(dec_b2, np.float32),
                          np.asarray(dec_w3, np.float32),
                          np.asarray(dec_b3, np.float32))
    in_maps = []
    for i in range(8):
        sh = slice(32 * i, 32 * (i + 1))
        m = dict(shared)
        m.update(_prep_core(np.asarray(inputs, np.float32)[sh],
                            np.asarray(labels)[sh]))
        in_maps.append(m)

    res = run_bass_kernel_spmd(nc, in_maps, core_ids=list(range(8)))

    clf = np.concatenate([res.results[i]["clf_d"] for i in range(8)], axis=0)
    recs = []
    for i in range(8):
        rt = res.results[i]["recT_d"]            # [128, 7, 32]
        r = rt.transpose(1, 0, 2).reshape(896, 32)[:784].T   # [32, 784]
        recs.append(r.reshape(32, 1, 28, 28))
    rec = np.concatenate(recs, axis=0)
    return clf.astype(np.float32), rec.astype(np.float32)


# revision 23
# speedup vs baseline: 1.0205x; 1.0055x over previous
"""CapsuleNet forward on 8 TRN2 NeuronCores, pure data-parallel over batch.

Per core (B=32): conv1(9x9 s1)+relu -> primary-caps conv(9x9 s2) -> squash ->
u_hat einsum (routing weights) -> 3 dynamic-routing iterations -> digit caps
-> classification norms + masked decoder MLP -> reconstruction.

Device layouts (see inline comments): conv stages keep channels on partitions
with free order (y, x, b); the routing phase keeps WU as [32j+b, g, cls, d]
where n = (8j + g//36)*36 + g%36 indexes the 1152 primary capsules.
"""
import sys
sys.path.insert(0, '/opt/trn_rl_repo')

import numpy as np
import ml_dtypes

import concourse.bass as bass
from concourse import bacc
import concourse.tile as tile
from concourse import mybir

f32 = mybir.dt.float32
bf16 = mybir.dt.bfloat16
npbf = ml_dtypes.bfloat16

B = 32            # per-core batch
NCLS = 10
DC = 16           # digit capsule dim
PC = 8            # primary capsule dim
NPRIM = 1152
CD = NCLS * DC    # 160

AF = mybir.ActivationFunctionType
ALU = mybir.AluOpType
AX = mybir.AxisListType

_CACHE = {}


def _build():
    nc = bacc.Bacc(None, target_bir_lowering=False)

    # ---- external inputs (per-core, host-prepped) ----
    d_imc = nc.dram_tensor("imc", [81, 400 * B], bf16, kind="ExternalInput")
    d_w1 = nc.dram_tensor("w1t", [81, 256], bf16, kind="ExternalInput")
    d_b1 = nc.dram_tensor("b1c", [128, 2], f32, kind="ExternalInput")
    d_pcw = nc.dram_tensor("pcwt", [81, 256, 256], bf16, kind="ExternalInput")
    d_pcb = nc.dram_tensor("pcbc", [128, 2], f32, kind="ExternalInput")
    d_rw = nc.dram_tensor("rw4", [4, 8, 288, 160], bf16, kind="ExternalInput")
    d_msk = nc.dram_tensor("mask32", [128, 32], bf16, kind="ExternalInput")
    d_ident = nc.dram_tensor("ident", [32, 32], bf16, kind="ExternalInput")
    d_oh = nc.dram_tensor("onehot", [32, 10], f32, kind="ExternalInput")
    d_w1d0 = nc.dram_tensor("w1d0", [128, 512], bf16, kind="ExternalInput")
    d_w1d1 = nc.dram_tensor("w1d1", [32, 512], bf16, kind="ExternalInput")
    d_w2d = nc.dram_tensor("w2d", [128, 4, 1024], bf16, kind="ExternalInput")
    d_w3d = nc.dram_tensor("w3d", [128, 8, 896], bf16, kind="ExternalInput")
    d_b1d = nc.dram_tensor("b1d", [128, 4], f32, kind="ExternalInput")
    d_b2d = nc.dram_tensor("b2d", [128, 8], f32, kind="ExternalInput")
    d_b3d = nc.dram_tensor("b3d", [128, 7], f32, kind="ExternalInput")

    # ---- outputs ----
    d_clf = nc.dram_tensor("clf_d", [32, 10], f32, kind="ExternalOutput")
    d_rec = nc.dram_tensor("recT_d", [128, 7, 32], f32, kind="ExternalOutput")

    # ---- internal scratch ----
    d_u = nc.dram_tensor("u_scr", [8, 4, 8, 1152], bf16, kind="Internal")

    with tile.TileContext(nc) as tc:
        _emit(nc, tc, locals())
    nc.compile()
    return nc


def _emit(nc, tc, d):
    import contextlib
    ctx = contextlib.ExitStack()
    with ctx:
        const = ctx.enter_context(tc.tile_pool(name="const", bufs=1))
        smallp = ctx.enter_context(tc.tile_pool(name="smallp", bufs=1))
        pcp = tc.alloc_tile_pool(name="pcp", bufs=1)
        xp = tc.alloc_tile_pool(name="xp", bufs=1)
        imcp = tc.alloc_tile_pool(name="imcp", bufs=1)
        psa_a = tc.alloc_tile_pool(name="psa_a", bufs=2, space="PSUM")

        # ============== stage A inputs first (startup latency) ==============
        imc = imcp.tile([81, 12800], bf16)
        nc.sync.dma_start(out=imc[:, 0:2560], in_=d["d_imc"][:, 0:2560])
        w1 = const.tile([81, 256], bf16)
        nc.sync.dma_start(out=w1, in_=d["d_w1"][:, :])
        b1 = const.tile([128, 2], f32)
        nc.sync.dma_start(out=b1, in_=d["d_b1"][:, :])
        for ic in range(1, 5):
            nc.sync.dma_start(out=imc[:, 2560 * ic:2560 * (ic + 1)],
                              in_=d["d_imc"][:, 2560 * ic:2560 * (ic + 1)])
        czero = const.tile([128, 1], f32)
        nc.vector.memset(czero, 0.0)
        ceps = const.tile([128, 1], f32)
        nc.vector.memset(ceps, 1e-7)
        nc.const_aps.aps[(f32, 0.0)] = czero[:, :]
        nc.const_aps.aps[(f32, 1e-7)] = ceps[:, :]
        pcb = const.tile([128, 2], f32)
        nc.sync.dma_start(out=pcb, in_=d["d_pcb"][:, :])
        msk = const.tile([128, 32], bf16)
        nc.sync.dma_start(out=msk, in_=d["d_msk"][:, :])
        x = [xp.tile([128, 12800], bf16, tag=f"x{c}", name=f"x{c}") for c in range(2)]
        for c in range(2):
            for t in range(25):
                pa = psa_a.tile([128, 512], f32)
                nc.tensor.matmul(pa[:, :], w1[:, 128 * c:128 * (c + 1)],
                                 imc[:, 512 * t:512 * (t + 1)],
                                 start=True, stop=True)
                # relu(x + b): alternate DVE / ACT
                o = x[c][:, 512 * t:512 * (t + 1)]
                if t % 2 == 0:
                    nc.scalar.activation(out=o, in_=pa[:, :], func=AF.Relu,
                                         bias=b1[:, c:c + 1], scale=1.0)
                else:
                    nc.vector.tensor_scalar(out=o, in0=pa[:, :],
                                            scalar1=b1[:, c:c + 1], scalar2=0.0,
                                            op0=ALU.add, op1=ALU.max)

        imcp.release()
        psa_a.release()

        # ============== stage B: primary caps conv (s2) ==============
        stream = tc.alloc_tile_pool(name="stream", bufs=2)
        psB = tc.alloc_tile_pool(name="psB", bufs=1, space="PSUM")
        # x[c] viewed (y20, x20, b32); out pc[c] [128, (yx36, b32)] f32
        pcs = [psB.tile([128, 384], f32, tag=f"pcs{i}", name=f"pcs{i}") for i in range(6)]
        TC = 9  # taps per stream chunk
        for tci in range(81 // TC):
            pw = stream.tile([128, TC, 2, 256], bf16, tag="pcw")
            for ci in range(2):
                nc.sync.dma_start(
                    out=pw[:, :, ci, :],
                    in_=d["d_pcw"][TC * tci:TC * (tci + 1),
                                   128 * ci:128 * (ci + 1), :]
                    .rearrange("t c o -> c t o"))
            for t in range(TC):
                tap = TC * tci + t
                ky, kx = divmod(tap, 9)
                for ci in range(2):
                    for co in range(2):
                        for nt in range(3):
                            rhs = bass.AP(
                                tensor=x[ci].tensor,
                                offset=x[ci].offset + (4 * nt + ky) * 640 + kx * 32,
                                ap=[x[ci].ap[0], [1280, 2], [64, 6], [1, 32]])
                            nc.tensor.matmul(
                                pcs[3 * co + nt][:, :],
                                pw[:, t, ci, 128 * co:128 * (co + 1)],
                                rhs,
                                start=(tap == 0 and ci == 0),
                                stop=(tap == 80 and ci == 1))
        pc = [pcp.tile([128, 1152], bf16, tag=f"pc{c}", name=f"pc{c}") for c in range(2)]
        with nc.allow_low_precision(reason="bf16 primary-caps activations"):
            for co in range(2):
                for nt in range(3):
                    nc.vector.tensor_scalar_add(
                        out=pc[co][:, 384 * nt:384 * (nt + 1)],
                        in0=pcs[3 * co + nt][:, :], scalar1=pcb[:, co:co + 1])

        stream.release()
        xp.release()

        # ============== squash -> u ==============
        sqp = tc.alloc_tile_pool(name="sqp", bufs=1)
        pcsq = [pcp.tile([128, 1152], bf16, tag=f"pcsq{c}", name=f"pcsq{c}") for c in range(2)]
        for c in range(2):
            nc.vector.tensor_tensor(out=pcsq[c], in0=pc[c], in1=pc[c], op=ALU.mult)
        psB.release()
        pssq = tc.alloc_tile_pool(name="pssq", bufs=1, space="PSUM")
        sps = [pssq.tile([32, 384], f32, tag=f"sps{i}", name=f"sps{i}") for i in range(3)]
        for nt in range(3):
            for c in range(2):
                nc.tensor.matmul(sps[nt][:, :], msk[:, :],
                                 pcsq[c][:, 384 * nt:384 * (nt + 1)],
                                 start=(c == 0), stop=(c == 1))
        t1 = sqp.tile([32, 1152], f32)
        t2 = sqp.tile([32, 1152], f32)
        for nt in range(3):
            sl = slice(384 * nt, 384 * (nt + 1))
            nc.scalar.activation(out=t1[:, sl], in_=sps[nt][:, :], func=AF.Sqrt,
                                 bias=1e-7, scale=1.0)
            nc.vector.tensor_scalar_add(out=t2[:, sl], in0=sps[nt][:, :],
                                        scalar1=1.0)
        r1 = sqp.tile([32, 1152], f32)
        nc.vector.reciprocal(out=r1, in_=t1)
        r2 = sqp.tile([32, 1152], f32)
        nc.vector.reciprocal(out=r2, in_=t2)
        fq = sqp.tile([32, 1152], f32)
        nc.vector.tensor_tensor(out=fq, in0=r1, in1=r2, op=ALU.mult)
        fqb = sqp.tile([32, 1152], bf16)
        for nt in range(3):
            sl = slice(384 * nt, 384 * (nt + 1))
            nc.vector.tensor_tensor(out=fqb[:, sl], in0=fq[:, sl],
                                    in1=sps[nt][:, :], op=ALU.mult)
        frep = sqp.tile([128, 1152], bf16)
        for q in range(4):
            nc.sync.dma_start(out=frep[32 * q:32 * (q + 1), :], in_=fqb[:, :])
        u = [sqp.tile([128, 1152], bf16, tag=f"u{c}", name=f"u{c}") for c in range(2)]
        for c in range(2):
            nc.vector.tensor_tensor(out=u[c], in0=pc[c], in1=frep, op=ALU.mult)

        # ============== u bounce through DRAM into strip layout ==============
        du = d["d_u"]
        for c in range(2):
            nc.sync.dma_start(
                out=du[4 * c:4 * (c + 1), :, :, :].rearrange("p j h f -> (p j h) f"),
                in_=u[c][:, :])
        sqp.release()
        pcp.release()
        pssq.release()
        wup = tc.alloc_tile_pool(name="wup", bufs=1)
        upadp = tc.alloc_tile_pool(name="upadp", bufs=1)
        rws = tc.alloc_tile_pool(name="rws", bufs=2)
        psE = tc.alloc_tile_pool(name="psE", bufs=3, space="PSUM")
        upad = upadp.tile([128, 288, 32], bf16)
        for j in range(4):
            for c in range(2):
                nc.sync.dma_start(
                    out=upad[32 * j + 4 * c:32 * j + 4 * (c + 1), :, :]
                    .rearrange("p (h y) b -> p h y b", h=8),
                    in_=du[4 * c:4 * (c + 1), j, :, :]
                    .rearrange("p h (y b) -> p h y b", y=36))

        # ============== WU einsum (strip-parallel small-K matmuls) ==========
        wu = wup.tile([128, 288, 160], bf16)
        for gc in range(8):
            rwt = rws.tile([128, 36, 160], bf16, tag="rwt")
            for j in range(4):
                nc.sync.dma_start(out=rwt[32 * j:32 * j + 8, :, :],
                                  in_=d["d_rw"][j, :, 36 * gc:36 * (gc + 1), :])
            for gg in range(6):
                pe = psE.tile([128, 2, 512], f32, tag="pe")
                for g6 in range(6):
                    gl = 6 * gg + g6
                    g = 36 * gc + gl
                    bk, g3 = divmod(g6, 3)
                    for j in range(4):
                        nc.tensor.matmul(
                            pe[32 * j:32 * (j + 1), bk, 160 * g3:160 * (g3 + 1)],
                            upad[32 * j:32 * j + 8, g, :],
                            rwt[32 * j:32 * j + 8, gl, :],
                            start=True, stop=True,
                            tile_position=(32 * j, 32 * j))
                o = wu[:, 36 * gc + 6 * gg:36 * gc + 6 * (gg + 1), :]
                o = o.rearrange("p a b -> p (a b)").rearrange(
                    "p (b x) -> p b x", b=2)
                i = bass.AP(tensor=pe.tensor, offset=pe.offset,
                            ap=[pe.ap[0], [512, 2], [1, 480]])
                if gg % 2 == 0:
                    nc.vector.tensor_copy(out=o, in_=i)
                else:
                    nc.scalar.copy(out=o, in_=i)

        rws.release()
        upadp.release()
        psE.release()

        # ============== routing ==============
        routp = tc.alloc_tile_pool(name="routp", bufs=1)
        dbuf = tc.alloc_tile_pool(name="dbuf", bufs=2)
        psR = tc.alloc_tile_pool(name="psR", bufs=2, space="PSUM")
        bij = routp.tile([128, 288, 10], f32)
        cbf = routp.tile([128, 288, 10], bf16)
        vrep = routp.tile([128, 160], bf16)
        tmpa = routp.tile([128, 48, 10], f32)
        tmpb = routp.tile([128, 48, 10], f32)
        v = smallp.tile([32, 160], f32)
        vpre = smallp.tile([32, 160], f32)
        vsb = smallp.tile([32, 160], bf16)
        vs = smallp.tile([32, 160], f32)
        sq = smallp.tile([32, 10], f32)
        w1q = smallp.tile([32, 10], f32)
        w2q = smallp.tile([32, 10], f32)
        fv = smallp.tile([32, 10], f32)
        exb = routp.tile([128, 288, 10], bf16)
        csum = routp.tile([128, 288], f32)
        crec = routp.tile([128, 288], bf16)



        def squash_v(scale):
            # reads vpre (sbuf); writes v (digit caps) and fv factors
            nc.scalar.activation(out=vs[:, :], in_=vpre[:, :], func=AF.Copy,
                                 bias=0.0, scale=scale)
            nc.vector.tensor_tensor(out=v[:, :], in0=vs, in1=vs, op=ALU.mult)
            vsqv = bass.AP(tensor=v.tensor, offset=v.offset,
                           ap=[v.ap[0], [1, 10], [10, 16]])
            nc.vector.tensor_reduce(out=sq[:, :], in_=vsqv,
                                    axis=AX.X, op=ALU.add)
            nc.scalar.activation(out=w1q[:, :], in_=sq[:, :], func=AF.Sqrt,
                                 bias=1e-7, scale=1.0)
            nc.vector.reciprocal(out=w1q, in_=w1q)
            nc.vector.tensor_scalar_add(out=w2q, in0=sq, scalar1=1.0)
            nc.vector.reciprocal(out=w2q, in_=w2q)
            nc.vector.tensor_tensor(out=fv, in0=w1q, in1=w2q, op=ALU.mult)
            nc.vector.tensor_tensor(out=fv, in0=fv, in1=sq, op=ALU.mult)
            fvb = bass.AP(tensor=fv.tensor, offset=fv.offset,
                          ap=[fv.ap[0], [0, 16], [1, 10]])
            nc.vector.tensor_tensor(
                out=v[:, :].rearrange("p (e c) -> p e c", e=16),
                in0=vs[:, :].rearrange("p (e c) -> p e c", e=16),
                in1=fvb, op=ALU.mult)

        def fold_squash(vpx, scale):
            fold = bass.AP(tensor=vpx.tensor, offset=vpx.offset,
                           ap=[vpx.ap[0], [1, 160], [160, 3]])
            nc.vector.tensor_reduce(out=vpre[:, :], in_=fold, axis=AX.X,
                                    op=ALU.add)
            squash_v(scale)

        def vrep_from_v(scale):
            nc.scalar.activation(out=vsb[:, :], in_=v[:, :], func=AF.Copy,
                                 bias=0.0, scale=scale)
            for q in range(4):
                nc.sync.dma_start(out=vrep[32 * q:32 * (q + 1), :],
                                  in_=vsb[:, :])

        def astep_chunk(gc, first):
            # b_ij[gc] (+)= c * sum_d WU*vrep  (vrep pre-scaled 0.1 when first)
            vb = bass.AP(tensor=vrep.tensor, offset=vrep.offset,
                         ap=[vrep.ap[0], [0, 48], [1, 160]])
            prod = dbuf.tile([128, 48, 160], bf16, tag="sprod", name="prod")
            nc.vector.tensor_tensor(
                out=prod[:, :, :], in0=wu[:, 48 * gc:48 * (gc + 1), :],
                in1=vb, op=ALU.mult)
            p4d = prod[:, :, :].rearrange("p g (e c) -> p g e c", e=16)
            t8 = dbuf.tile([128, 48, 8, 10], bf16, tag="tree", name="t8")
            nc.vector.tensor_tensor(out=t8, in0=p4d[:, :, 0:8, :],
                                    in1=p4d[:, :, 8:16, :], op=ALU.add)
            t4 = dbuf.tile([128, 48, 4, 10], bf16, tag="tree", name="t4")
            nc.vector.tensor_tensor(out=t4, in0=t8[:, :, 0:4, :],
                                    in1=t8[:, :, 4:8, :], op=ALU.add)
            t2 = dbuf.tile([128, 48, 2, 10], bf16, tag="tree", name="t2")
            nc.vector.tensor_tensor(out=t2, in0=t4[:, :, 0:2, :],
                                    in1=t4[:, :, 2:4, :], op=ALU.add)
            dst = (bij[:, 48 * gc:48 * (gc + 1), :] if first
                   else tmpa[:, :, :])
            nc.vector.tensor_tensor(out=dst, in0=t2[:, :, 0, :],
                                    in1=t2[:, :, 1, :], op=ALU.add)
            if not first:
                cb = cbf[:, 48 * gc:48 * (gc + 1), :]
                nc.vector.tensor_tensor(out=tmpb[:, :, :], in0=tmpa[:, :, :],
                                        in1=cb, op=ALU.mult)
                nc.vector.tensor_tensor(
                    out=bij[:, 48 * gc:48 * (gc + 1), :],
                    in0=bij[:, 48 * gc:48 * (gc + 1), :],
                    in1=tmpb[:, :, :], op=ALU.add)

        def softmax_chunk(gc):
            sl = slice(48 * gc, 48 * (gc + 1))
            nc.scalar.activation(
                out=exb[:, sl, :].rearrange("p a b -> p (a b)"),
                in_=bij[:, sl, :].rearrange("p a b -> p (a b)"),
                func=AF.Exp, bias=0.0, scale=1.0)
            nc.vector.tensor_reduce(out=csum[:, sl], in_=exb[:, sl, :],
                                    axis=AX.X, op=ALU.add)
            with nc.allow_low_precision(reason="bf16 softmax reciprocal"):
                nc.vector.reciprocal(out=crec[:, sl], in_=csum[:, sl])
            cr = bass.AP(tensor=crec.tensor, offset=crec.offset + 48 * gc,
                         ap=[crec.ap[0], [1, 48], [0, 10]])
            nc.vector.tensor_tensor(out=cbf[:, sl, :], in0=exb[:, sl, :],
                                    in1=cr, op=ALU.mult)

        def smult_mm_chunk(vpx, gc, use_c):
            if use_c:
                cb = cbf[:, 48 * gc:48 * (gc + 1), :]
                cb = bass.AP(tensor=cb.tensor, offset=cb.offset,
                             ap=[cb.ap[0], [10, 48], [0, 16], [1, 10]])
                sprod = dbuf.tile([128, 48, 160], bf16, tag="sprod",
                                  name="sprod")
                nc.vector.tensor_tensor(
                    out=sprod[:, :, :].rearrange("p g (e c) -> p g e c", e=16),
                    in0=wu[:, 48 * gc:48 * (gc + 1), :]
                    .rearrange("p g (e c) -> p g e c", e=16),
                    in1=cb, op=ALU.mult)
                srcs = sprod
            else:
                srcs = wu[:, 48 * gc:48 * (gc + 1), :]
            for tt in range(16):
                rhs = srcs[:, 3 * tt:3 * (tt + 1), :].rearrange(
                    "p a b -> p (a b)")
                nc.tensor.matmul(vpx[:, :], msk[:, :], rhs,
                                 start=(gc == 0 and tt == 0),
                                 stop=(gc == 5 and tt == 15))

        # ---- iter 0: v0 from raw WU ----
        vpx0 = psR.tile([32, 480], f32, tag="vpx", name="vpx0")
        for gc in range(6):
            smult_mm_chunk(vpx0, gc, use_c=False)
        fold_squash(vpx0, 0.1)
        vrep_from_v(0.1)
        # ---- astep0 + softmax1 + smult1 pipelined per chunk ----
        vpx1 = psR.tile([32, 480], f32, tag="vpx", name="vpx1")
        for gc in range(6):
            astep_chunk(gc, first=True)
            softmax_chunk(gc)
            smult_mm_chunk(vpx1, gc, use_c=True)
        fold_squash(vpx1, 1.0)
        vrep_from_v(1.0)
        # ---- astep1 + softmax2 + smult2 pipelined per chunk ----
        vpx2 = psR.tile([32, 480], f32, tag="vpx", name="vpx2")
        for gc in range(6):
            astep_chunk(gc, first=False)
            softmax_chunk(gc)
            smult_mm_chunk(vpx2, gc, use_c=True)
        fold_squash(vpx2, 1.0)

        # ============== clf output: ||v_final|| = sqrt(sq) * fv ==============
        clf = smallp.tile([32, 10], f32)
        nc.scalar.activation(out=clf, in_=sq[:, :], func=AF.Sqrt,
                             bias=0.0, scale=1.0)
        nc.vector.tensor_tensor(out=clf, in0=clf, in1=fv, op=ALU.mult)
        nc.sync.dma_start(out=d["d_clf"][:, :], in_=clf)

        dbuf.release()
        routp.release()
        wup.release()
        psR.release()

        # ============== decoder ==============
        decp = tc.alloc_tile_pool(name="decp", bufs=1)
        psD = tc.alloc_tile_pool(name="psD", bufs=2, space="PSUM")
        oh = decp.tile([32, 10], f32)
        nc.sync.dma_start(out=oh, in_=d["d_oh"][:, :])
        ident = decp.tile([32, 32], bf16)
        nc.sync.dma_start(out=ident, in_=d["d_ident"][:, :])
        wd10 = decp.tile([128, 512], bf16)
        nc.sync.dma_start(out=wd10, in_=d["d_w1d0"][:, :])
        wd11 = decp.tile([32, 512], bf16)
        nc.sync.dma_start(out=wd11, in_=d["d_w1d1"][:, :])
        wd2 = decp.tile([128, 4, 1024], bf16)
        nc.sync.dma_start(out=wd2, in_=d["d_w2d"][:, :, :])
        wd3 = decp.tile([128, 8, 896], bf16)
        nc.sync.dma_start(out=wd3, in_=d["d_w3d"][:, :, :])
        bd1 = decp.tile([128, 4], f32)
        nc.sync.dma_start(out=bd1, in_=d["d_b1d"][:, :])
        bd2 = decp.tile([128, 8], f32)
        nc.sync.dma_start(out=bd2, in_=d["d_b2d"][:, :])
        bd3 = decp.tile([128, 7], f32)
        nc.sync.dma_start(out=bd3, in_=d["d_b3d"][:, :])

        mskd = decp.tile([32, 160], bf16)
        ohb = bass.AP(tensor=oh.tensor, offset=oh.offset,
                      ap=[oh.ap[0], [0, 16], [1, 10]])
        nc.vector.tensor_tensor(
            out=mskd[:, :].rearrange("p (e c) -> p e c", e=16),
            in0=v[:, :].rearrange("p (e c) -> p e c", e=16),
            in1=ohb, op=ALU.mult)
        # transpose masked v -> h_T chunks [128, 32] + [32, 32]
        ph0 = psD.tile([128, 32], bf16, tag="ph0")
        nc.tensor.transpose(ph0[:, :], mskd[:, 0:128], ident[:, :])
        ph1 = psD.tile([32, 32], bf16, tag="ph1")
        nc.tensor.transpose(ph1[:, :], mskd[:, 128:160], ident[:, :])
        h0 = decp.tile([128, 32], bf16)
        nc.vector.tensor_copy(out=h0, in_=ph0[:, :])
        h1 = decp.tile([32, 32], bf16)
        nc.vector.tensor_copy(out=h1, in_=ph1[:, :])

        hd1 = decp.tile([128, 4, 32], bf16)
        for mt in range(4):
            pd = psD.tile([128, 32], f32, tag="pd")
            nc.tensor.matmul(pd[:, :], wd10[:, 128 * mt:128 * (mt + 1)],
                             h0[:, :], start=True, stop=False)
            nc.tensor.matmul(pd[:, :], wd11[:, 128 * mt:128 * (mt + 1)],
                             h1[:, :], start=False, stop=True)
            nc.scalar.activation(out=hd1[:, mt, :], in_=pd[:, :], func=AF.Relu,
                                 bias=bd1[:, mt:mt + 1], scale=1.0)
        hd2 = decp.tile([128, 8, 32], bf16)
        for mt in range(8):
            pd = psD.tile([128, 32], f32, tag="pd")
            for kc in range(4):
                nc.tensor.matmul(pd[:, :], wd2[:, kc, 128 * mt:128 * (mt + 1)],
                                 hd1[:, kc, :], start=(kc == 0), stop=(kc == 3))
            nc.scalar.activation(out=hd2[:, mt, :], in_=pd[:, :], func=AF.Relu,
                                 bias=bd2[:, mt:mt + 1], scale=1.0)
        rec = decp.tile([128, 7, 32], f32)
        for mt in range(7):
            pd = psD.tile([128, 32], f32, tag="pd")
            for kc in range(8):
                nc.tensor.matmul(pd[:, :], wd3[:, kc, 128 * mt:128 * (mt + 1)],
                                 hd2[:, kc, :], start=(kc == 0), stop=(kc == 7))
            nc.scalar.activation(out=rec[:, mt, :], in_=pd[:, :], func=AF.Sigmoid,
                                 bias=bd3[:, mt:mt + 1], scale=1.0)
        nc.sync.dma_start(out=d["d_rec"][:, :, :], in_=rec)
        decp.release()
        psD.release()


def _prep_shared(conv1_w, conv1_b, pc_w, pc_b, routing_weights,
                 dec_w1, dec_b1, dec_w2, dec_b2, dec_w3, dec_b3):
    s = {}
    s["w1t"] = np.ascontiguousarray(
        conv1_w.transpose(2, 3, 1, 0).reshape(81, 256)).astype(npbf)
    s["b1c"] = np.ascontiguousarray(conv1_b.reshape(2, 128).T).astype(np.float32)
    s["pcwt"] = np.ascontiguousarray(
        pc_w.transpose(2, 3, 1, 0).reshape(81, 256, 256)).astype(npbf)
    s["pcbc"] = np.ascontiguousarray(pc_b.reshape(2, 128).T).astype(np.float32)
    # rw4[j, p, g=(h,yx), cd]: RW[n, cls, d, p], n = (8j+h)*36+yx
    R = routing_weights.reshape(4, 8, 36, 10, 16, 8)  # [j, h, yx, cls, d, p]
    s["rw4"] = np.ascontiguousarray(
        R.transpose(0, 5, 1, 2, 4, 3).reshape(4, 8, 288, 160)).astype(npbf)
    s["mask32"] = ((np.arange(128)[:, None] % 32) ==
                   np.arange(32)[None, :]).astype(npbf)
    s["ident"] = np.eye(32).astype(npbf)
    w1r = dec_w1.reshape(10, 16, 512).transpose(1, 0, 2).reshape(160, 512)
    s["w1d0"] = np.ascontiguousarray(w1r[:128]).astype(npbf)
    s["w1d1"] = np.ascontiguousarray(w1r[128:]).astype(npbf)
    s["w2d"] = np.ascontiguousarray(
        dec_w2.reshape(4, 128, 1024).transpose(1, 0, 2)).astype(npbf)
    w3p = np.concatenate([dec_w3, np.zeros((1024, 112), dec_w3.dtype)], axis=1)
    s["w3d"] = np.ascontiguousarray(
        w3p.reshape(8, 128, 896).transpose(1, 0, 2)).astype(npbf)
    s["b1d"] = np.ascontiguousarray(dec_b1.reshape(4, 128).T).astype(np.float32)
    s["b2d"] = np.ascontiguousarray(dec_b2.reshape(8, 128).T).astype(np.float32)
    b3p = np.concatenate([dec_b3, np.zeros(112, dec_b3.dtype)])
    s["b3d"] = np.ascontiguousarray(b3p.reshape(7, 128).T).astype(np.float32)
    return s


def _prep_core(inputs_sh, labels_sh):
    m = {}
    arr = np.asarray(inputs_sh[:, 0], np.float32)          # [32, 28, 28]
    A = np.empty((9, 9, 20, 20, 32), np.float32)
    for ky in range(9):
        for kx in range(9):
            A[ky, kx] = arr[:, ky:ky + 20, kx:kx + 20].transpose(1, 2, 0)
    m["imc"] = A.reshape(81, 400 * 32).astype(npbf)
    oh = np.zeros((32, 10), np.float32)
    oh[np.arange(32), np.asarray(labels_sh)] = 1.0
    m["onehot"] = oh
    return m


def kernel(inputs, labels, conv1_w, conv1_b, pc_w, pc_b, routing_weights,
           dec_w1, dec_b1, dec_w2, dec_b2, dec_w3, dec_b3):
    from concourse.bass_utils import run_bass_kernel_spmd
    if "nc" not in _CACHE:
        _CACHE["nc"] = _build()
    nc = _CACHE["nc"]

    shared = _prep_shared(np.asarray(conv1_w, np.float32),
                          np.asarray(conv1_b, np.float32),
                          np.asarray(pc_w, np.float32),
                          np.asarray(pc_b, np.float32),
                          np.asarray(routing_weights, np.float32),
                          np.asarray(dec_w1, np.float32),
                          np.asarray(dec_b1, np.float32),
                          np.asarray(dec_w2, np.float32),
                          np.asarray(dec_b2, np.float32),
                          np.asarray(dec_w3, np.float32),
                          np.asarray(dec_b3, np.float32))
    in_maps = []
    for i in range(8):
        sh = slice(32 * i, 32 * (i + 1))
        m = dict(shared)
        m.update(_prep_core(np.asarray(inputs, np.float32)[sh],
                            np.asarray(labels)[sh]))
        in_maps.append(m)

    res = run_bass_kernel_spmd(nc, in_maps, core_ids=list(range(8)))

    clf = np.concatenate([res.results[i]["clf_d"] for i in range(8)], axis=0)
    recs = []
    for i in range(8):
        rt = res.results[i]["recT_d"]            # [128, 7, 32]
        r = rt.transpose(1, 0, 2).reshape(896, 32)[:784].T   # [32, 784]
        recs.append(r.reshape(32, 1, 28, 28))
    rec = np.concatenate(recs, axis=0)
    return clf.astype(np.float32), rec.astype(np.float32)


# revision 24
# speedup vs baseline: 1.0280x; 1.0074x over previous
"""CapsuleNet forward on 8 TRN2 NeuronCores, pure data-parallel over batch.

Per core (B=32): conv1(9x9 s1)+relu -> primary-caps conv(9x9 s2) -> squash ->
u_hat einsum (routing weights) -> 3 dynamic-routing iterations -> digit caps
-> classification norms + masked decoder MLP -> reconstruction.

Device layouts (see inline comments): conv stages keep channels on partitions
with free order (y, x, b); the routing phase keeps WU as [32j+b, g, cls, d]
where n = (8j + g//36)*36 + g%36 indexes the 1152 primary capsules.
"""
import sys
sys.path.insert(0, '/opt/trn_rl_repo')

import numpy as np
import ml_dtypes

import concourse.bass as bass
from concourse import bacc
import concourse.tile as tile
from concourse import mybir

f32 = mybir.dt.float32
bf16 = mybir.dt.bfloat16
npbf = ml_dtypes.bfloat16

B = 32            # per-core batch
NCLS = 10
DC = 16           # digit capsule dim
PC = 8            # primary capsule dim
NPRIM = 1152
CD = NCLS * DC    # 160

AF = mybir.ActivationFunctionType
ALU = mybir.AluOpType
AX = mybir.AxisListType

_CACHE = {}


def _build():
    nc = bacc.Bacc(None, target_bir_lowering=False)

    # ---- external inputs (per-core, host-prepped) ----
    d_imc = nc.dram_tensor("imc", [81, 400 * B], bf16, kind="ExternalInput")
    d_w1 = nc.dram_tensor("w1t", [81, 256], bf16, kind="ExternalInput")
    d_b1 = nc.dram_tensor("b1c", [128, 2], f32, kind="ExternalInput")
    d_pcw = nc.dram_tensor("pcwt", [81, 256, 256], bf16, kind="ExternalInput")
    d_pcb = nc.dram_tensor("pcbc", [128, 2], f32, kind="ExternalInput")
    d_rw = nc.dram_tensor("rw4", [4, 8, 288, 160], bf16, kind="ExternalInput")
    d_msk = nc.dram_tensor("mask32", [128, 32], bf16, kind="ExternalInput")
    d_ident = nc.dram_tensor("ident", [32, 32], bf16, kind="ExternalInput")
    d_oh = nc.dram_tensor("onehot", [32, 10], f32, kind="ExternalInput")
    d_w1d0 = nc.dram_tensor("w1d0", [128, 512], bf16, kind="ExternalInput")
    d_w1d1 = nc.dram_tensor("w1d1", [32, 512], bf16, kind="ExternalInput")
    d_w2d = nc.dram_tensor("w2d", [128, 4, 1024], bf16, kind="ExternalInput")
    d_w3d = nc.dram_tensor("w3d", [128, 8, 896], bf16, kind="ExternalInput")
    d_b1d = nc.dram_tensor("b1d", [128, 4], f32, kind="ExternalInput")
    d_b2d = nc.dram_tensor("b2d", [128, 8], f32, kind="ExternalInput")
    d_b3d = nc.dram_tensor("b3d", [128, 7], f32, kind="ExternalInput")

    # ---- outputs ----
    d_clf = nc.dram_tensor("clf_d", [32, 10], f32, kind="ExternalOutput")
    d_rec = nc.dram_tensor("recT_d", [128, 7, 32], f32, kind="ExternalOutput")

    # ---- internal scratch ----
    d_u = nc.dram_tensor("u_scr", [8, 4, 8, 1152], bf16, kind="Internal")

    with tile.TileContext(nc) as tc:
        _emit(nc, tc, locals())
    nc.compile()
    return nc


def _emit(nc, tc, d):
    import contextlib
    ctx = contextlib.ExitStack()
    with ctx:
        const = ctx.enter_context(tc.tile_pool(name="const", bufs=1))
        smallp = ctx.enter_context(tc.tile_pool(name="smallp", bufs=1))
        pcp = tc.alloc_tile_pool(name="pcp", bufs=1)
        xp = tc.alloc_tile_pool(name="xp", bufs=1)
        imcp = tc.alloc_tile_pool(name="imcp", bufs=1)
        psa_a = tc.alloc_tile_pool(name="psa_a", bufs=2, space="PSUM")

        # ============== stage A inputs first (startup latency) ==============
        imc = imcp.tile([81, 12800], bf16)
        nc.sync.dma_start(out=imc[:, 0:2560], in_=d["d_imc"][:, 0:2560])
        w1 = const.tile([81, 256], bf16)
        nc.sync.dma_start(out=w1, in_=d["d_w1"][:, :])
        b1 = const.tile([128, 2], f32)
        nc.sync.dma_start(out=b1, in_=d["d_b1"][:, :])
        for ic in range(1, 5):
            nc.sync.dma_start(out=imc[:, 2560 * ic:2560 * (ic + 1)],
                              in_=d["d_imc"][:, 2560 * ic:2560 * (ic + 1)])
        czero = const.tile([128, 1], f32)
        nc.vector.memset(czero, 0.0)
        ceps = const.tile([128, 1], f32)
        nc.vector.memset(ceps, 1e-7)
        nc.const_aps.aps[(f32, 0.0)] = czero[:, :]
        nc.const_aps.aps[(f32, 1e-7)] = ceps[:, :]
        pcb = const.tile([128, 2], f32)
        nc.sync.dma_start(out=pcb, in_=d["d_pcb"][:, :])
        msk = const.tile([128, 32], bf16)
        nc.sync.dma_start(out=msk, in_=d["d_msk"][:, :])
        x = [xp.tile([128, 12800], bf16, tag=f"x{c}", name=f"x{c}") for c in range(2)]
        for c in range(2):
            for t in range(25):
                pa = psa_a.tile([128, 512], f32)
                nc.tensor.matmul(pa[:, :], w1[:, 128 * c:128 * (c + 1)],
                                 imc[:, 512 * t:512 * (t + 1)],
                                 start=True, stop=True)
                # relu(x + b): alternate DVE / ACT
                o = x[c][:, 512 * t:512 * (t + 1)]
                if t % 2 == 0:
                    nc.scalar.activation(out=o, in_=pa[:, :], func=AF.Relu,
                                         bias=b1[:, c:c + 1], scale=1.0)
                else:
                    nc.vector.tensor_scalar(out=o, in0=pa[:, :],
                                            scalar1=b1[:, c:c + 1], scalar2=0.0,
                                            op0=ALU.add, op1=ALU.max)

        imcp.release()
        psa_a.release()

        # ============== stage B: primary caps conv (s2) ==============
        stream = tc.alloc_tile_pool(name="stream", bufs=2)
        psB = tc.alloc_tile_pool(name="psB", bufs=1, space="PSUM")
        # x[c] viewed (y20, x20, b32); out pc[c] [128, (yx36, b32)] f32
        pcs = [psB.tile([128, 384], f32, tag=f"pcs{i}", name=f"pcs{i}") for i in range(6)]
        TC = 9  # taps per stream chunk
        for tci in range(81 // TC):
            pw = stream.tile([128, TC, 2, 256], bf16, tag="pcw")
            for ci in range(2):
                nc.sync.dma_start(
                    out=pw[:, :, ci, :],
                    in_=d["d_pcw"][TC * tci:TC * (tci + 1),
                                   128 * ci:128 * (ci + 1), :]
                    .rearrange("t c o -> c t o"))
            for t in range(TC):
                tap = TC * tci + t
                ky, kx = divmod(tap, 9)
                for ci in range(2):
                    for co in range(2):
                        for nt in range(3):
                            rhs = bass.AP(
                                tensor=x[ci].tensor,
                                offset=x[ci].offset + (4 * nt + ky) * 640 + kx * 32,
                                ap=[x[ci].ap[0], [1280, 2], [64, 6], [1, 32]])
                            nc.tensor.matmul(
                                pcs[3 * co + nt][:, :],
                                pw[:, t, ci, 128 * co:128 * (co + 1)],
                                rhs,
                                start=(tap == 0 and ci == 0),
                                stop=(tap == 80 and ci == 1))
        pc = [pcp.tile([128, 1152], bf16, tag=f"pc{c}", name=f"pc{c}") for c in range(2)]
        with nc.allow_low_precision(reason="bf16 primary-caps activations"):
            for co in range(2):
                for nt in range(3):
                    nc.vector.tensor_scalar_add(
                        out=pc[co][:, 384 * nt:384 * (nt + 1)],
                        in0=pcs[3 * co + nt][:, :], scalar1=pcb[:, co:co + 1])

        stream.release()
        xp.release()

        # ============== squash -> u ==============
        sqp = tc.alloc_tile_pool(name="sqp", bufs=1)
        pcsq = [pcp.tile([128, 1152], bf16, tag=f"pcsq{c}", name=f"pcsq{c}") for c in range(2)]
        for c in range(2):
            nc.vector.tensor_tensor(out=pcsq[c], in0=pc[c], in1=pc[c], op=ALU.mult)
        psB.release()
        pssq = tc.alloc_tile_pool(name="pssq", bufs=1, space="PSUM")
        sps = [pssq.tile([32, 384], f32, tag=f"sps{i}", name=f"sps{i}") for i in range(3)]
        for nt in range(3):
            for c in range(2):
                nc.tensor.matmul(sps[nt][:, :], msk[:, :],
                                 pcsq[c][:, 384 * nt:384 * (nt + 1)],
                                 start=(c == 0), stop=(c == 1))
        t1 = sqp.tile([32, 1152], f32)
        t2 = sqp.tile([32, 1152], f32)
        for nt in range(3):
            sl = slice(384 * nt, 384 * (nt + 1))
            nc.scalar.activation(out=t1[:, sl], in_=sps[nt][:, :], func=AF.Sqrt,
                                 bias=1e-7, scale=1.0)
            nc.vector.tensor_scalar_add(out=t2[:, sl], in0=sps[nt][:, :],
                                        scalar1=1.0)
        r1 = sqp.tile([32, 1152], f32)
        nc.vector.reciprocal(out=r1, in_=t1)
        r2 = sqp.tile([32, 1152], f32)
        nc.vector.reciprocal(out=r2, in_=t2)
        fq = sqp.tile([32, 1152], f32)
        nc.vector.tensor_tensor(out=fq, in0=r1, in1=r2, op=ALU.mult)
        fqb = sqp.tile([32, 1152], bf16)
        for nt in range(3):
            sl = slice(384 * nt, 384 * (nt + 1))
            nc.vector.tensor_tensor(out=fqb[:, sl], in0=fq[:, sl],
                                    in1=sps[nt][:, :], op=ALU.mult)
        frep = sqp.tile([128, 1152], bf16)
        for q in range(4):
            nc.sync.dma_start(out=frep[32 * q:32 * (q + 1), :], in_=fqb[:, :])
        u = [sqp.tile([128, 1152], bf16, tag=f"u{c}", name=f"u{c}") for c in range(2)]
        for c in range(2):
            nc.vector.tensor_tensor(out=u[c], in0=pc[c], in1=frep, op=ALU.mult)

        # ============== u bounce through DRAM into strip layout ==============
        du = d["d_u"]
        for c in range(2):
            nc.sync.dma_start(
                out=du[4 * c:4 * (c + 1), :, :, :].rearrange("p j h f -> (p j h) f"),
                in_=u[c][:, :])
        sqp.release()
        pcp.release()
        pssq.release()
        wup = tc.alloc_tile_pool(name="wup", bufs=1)
        upadp = tc.alloc_tile_pool(name="upadp", bufs=1)
        rws = tc.alloc_tile_pool(name="rws", bufs=3)
        psE = tc.alloc_tile_pool(name="psE", bufs=3, space="PSUM")
        upad = upadp.tile([128, 288, 32], bf16)
        for j in range(4):
            for c in range(2):
                nc.sync.dma_start(
                    out=upad[32 * j + 4 * c:32 * j + 4 * (c + 1), :, :]
                    .rearrange("p (h y) b -> p h y b", h=8),
                    in_=du[4 * c:4 * (c + 1), j, :, :]
                    .rearrange("p h (y b) -> p h y b", y=36))

        # ============== WU einsum (strip-parallel small-K matmuls) ==========
        wu = wup.tile([128, 288, 160], bf16)
        for gc in range(8):
            rwt = rws.tile([128, 36, 160], bf16, tag="rwt")
            for j in range(4):
                nc.sync.dma_start(out=rwt[32 * j:32 * j + 8, :, :],
                                  in_=d["d_rw"][j, :, 36 * gc:36 * (gc + 1), :])
            for gg in range(6):
                pe = psE.tile([128, 2, 512], f32, tag="pe")
                for g6 in range(6):
                    gl = 6 * gg + g6
                    g = 36 * gc + gl
                    bk, g3 = divmod(g6, 3)
                    for j in range(4):
                        nc.tensor.matmul(
                            pe[32 * j:32 * (j + 1), bk, 160 * g3:160 * (g3 + 1)],
                            upad[32 * j:32 * j + 8, g, :],
                            rwt[32 * j:32 * j + 8, gl, :],
                            start=True, stop=True,
                            tile_position=(32 * j, 32 * j))
                o = wu[:, 36 * gc + 6 * gg:36 * gc + 6 * (gg + 1), :]
                o = o.rearrange("p a b -> p (a b)").rearrange(
                    "p (b x) -> p b x", b=2)
                i = bass.AP(tensor=pe.tensor, offset=pe.offset,
                            ap=[pe.ap[0], [512, 2], [1, 480]])
                if gg % 2 == 0:
                    nc.vector.tensor_copy(out=o, in_=i)
                else:
                    nc.scalar.copy(out=o, in_=i)

        rws.release()
        upadp.release()
        psE.release()

        # ============== routing ==============
        routp = tc.alloc_tile_pool(name="routp", bufs=1)
        dbuf = tc.alloc_tile_pool(name="dbuf", bufs=2)
        psR = tc.alloc_tile_pool(name="psR", bufs=2, space="PSUM")
        bij = routp.tile([128, 288, 10], f32)
        cbf = routp.tile([128, 288, 10], bf16)
        vrep = routp.tile([128, 160], bf16)
        tmpa = routp.tile([128, 48, 10], f32)
        tmpb = routp.tile([128, 48, 10], f32)
        v = smallp.tile([32, 160], f32)
        vpre = smallp.tile([32, 160], f32)
        vsb = smallp.tile([32, 160], bf16)
        vs = smallp.tile([32, 160], f32)
        sq = smallp.tile([32, 10], f32)
        w1q = smallp.tile([32, 10], f32)
        w2q = smallp.tile([32, 10], f32)
        fv = smallp.tile([32, 10], f32)
        exb = routp.tile([128, 288, 10], bf16)
        csum = routp.tile([128, 288], f32)
        crec = routp.tile([128, 288], bf16)



        def squash_v(scale):
            # reads vpre (sbuf); writes v (digit caps) and fv factors
            nc.scalar.activation(out=vs[:, :], in_=vpre[:, :], func=AF.Copy,
                                 bias=0.0, scale=scale)
            nc.vector.tensor_tensor(out=v[:, :], in0=vs, in1=vs, op=ALU.mult)
            vsqv = bass.AP(tensor=v.tensor, offset=v.offset,
                           ap=[v.ap[0], [1, 10], [10, 16]])
            nc.vector.tensor_reduce(out=sq[:, :], in_=vsqv,
                                    axis=AX.X, op=ALU.add)
            nc.scalar.activation(out=w1q[:, :], in_=sq[:, :], func=AF.Sqrt,
                                 bias=1e-7, scale=1.0)
            nc.vector.reciprocal(out=w1q, in_=w1q)
            nc.vector.tensor_scalar_add(out=w2q, in0=sq, scalar1=1.0)
            nc.vector.reciprocal(out=w2q, in_=w2q)
            nc.vector.tensor_tensor(out=fv, in0=w1q, in1=w2q, op=ALU.mult)
            nc.vector.tensor_tensor(out=fv, in0=fv, in1=sq, op=ALU.mult)
            fvb = bass.AP(tensor=fv.tensor, offset=fv.offset,
                          ap=[fv.ap[0], [0, 16], [1, 10]])
            nc.vector.tensor_tensor(
                out=v[:, :].rearrange("p (e c) -> p e c", e=16),
                in0=vs[:, :].rearrange("p (e c) -> p e c", e=16),
                in1=fvb, op=ALU.mult)

        def fold_squash(vpx, scale):
            fold = bass.AP(tensor=vpx.tensor, offset=vpx.offset,
                           ap=[vpx.ap[0], [1, 160], [160, 3]])
            nc.vector.tensor_reduce(out=vpre[:, :], in_=fold, axis=AX.X,
                                    op=ALU.add)
            squash_v(scale)

        def vrep_from_v(scale):
            nc.scalar.activation(out=vsb[:, :], in_=v[:, :], func=AF.Copy,
                                 bias=0.0, scale=scale)
            for q in range(4):
                nc.sync.dma_start(out=vrep[32 * q:32 * (q + 1), :],
                                  in_=vsb[:, :])

        def astep_chunk(gc, first):
            # b_ij[gc] (+)= c * sum_d WU*vrep  (vrep pre-scaled 0.1 when first)
            vb = bass.AP(tensor=vrep.tensor, offset=vrep.offset,
                         ap=[vrep.ap[0], [0, 48], [1, 160]])
            prod = dbuf.tile([128, 48, 160], bf16, tag="sprod", name="prod")
            nc.vector.tensor_tensor(
                out=prod[:, :, :], in0=wu[:, 48 * gc:48 * (gc + 1), :],
                in1=vb, op=ALU.mult)
            p4d = prod[:, :, :].rearrange("p g (e c) -> p g e c", e=16)
            t8 = dbuf.tile([128, 48, 8, 10], bf16, tag="tree", name="t8")
            nc.vector.tensor_tensor(out=t8, in0=p4d[:, :, 0:8, :],
                                    in1=p4d[:, :, 8:16, :], op=ALU.add)
            t4 = dbuf.tile([128, 48, 4, 10], bf16, tag="tree", name="t4")
            nc.vector.tensor_tensor(out=t4, in0=t8[:, :, 0:4, :],
                                    in1=t8[:, :, 4:8, :], op=ALU.add)
            t2 = dbuf.tile([128, 48, 2, 10], bf16, tag="tree", name="t2")
            nc.vector.tensor_tensor(out=t2, in0=t4[:, :, 0:2, :],
                                    in1=t4[:, :, 2:4, :], op=ALU.add)
            dst = (bij[:, 48 * gc:48 * (gc + 1), :] if first
                   else tmpa[:, :, :])
            nc.vector.tensor_tensor(out=dst, in0=t2[:, :, 0, :],
                                    in1=t2[:, :, 1, :], op=ALU.add)
            if not first:
                cb = cbf[:, 48 * gc:48 * (gc + 1), :]
                nc.vector.tensor_tensor(out=tmpb[:, :, :], in0=tmpa[:, :, :],
                                        in1=cb, op=ALU.mult)
                nc.vector.tensor_tensor(
                    out=bij[:, 48 * gc:48 * (gc + 1), :],
                    in0=bij[:, 48 * gc:48 * (gc + 1), :],
                    in1=tmpb[:, :, :], op=ALU.add)

        def softmax_chunk(gc):
            sl = slice(48 * gc, 48 * (gc + 1))
            nc.scalar.activation(
                out=exb[:, sl, :].rearrange("p a b -> p (a b)"),
                in_=bij[:, sl, :].rearrange("p a b -> p (a b)"),
                func=AF.Exp, bias=0.0, scale=1.0)
            nc.vector.tensor_reduce(out=csum[:, sl], in_=exb[:, sl, :],
                                    axis=AX.X, op=ALU.add)
            with nc.allow_low_precision(reason="bf16 softmax reciprocal"):
                nc.vector.reciprocal(out=crec[:, sl], in_=csum[:, sl])
            cr = bass.AP(tensor=crec.tensor, offset=crec.offset + 48 * gc,
                         ap=[crec.ap[0], [1, 48], [0, 10]])
            nc.vector.tensor_tensor(out=cbf[:, sl, :], in0=exb[:, sl, :],
                                    in1=cr, op=ALU.mult)

        def smult_mm_chunk(vpx, gc, use_c):
            if use_c:
                cb = cbf[:, 48 * gc:48 * (gc + 1), :]
                cb = bass.AP(tensor=cb.tensor, offset=cb.offset,
                             ap=[cb.ap[0], [10, 48], [0, 16], [1, 10]])
                sprod = dbuf.tile([128, 48, 160], bf16, tag="sprod",
                                  name="sprod")
                nc.vector.tensor_tensor(
                    out=sprod[:, :, :].rearrange("p g (e c) -> p g e c", e=16),
                    in0=wu[:, 48 * gc:48 * (gc + 1), :]
                    .rearrange("p g (e c) -> p g e c", e=16),
                    in1=cb, op=ALU.mult)
                srcs = sprod
            else:
                srcs = wu[:, 48 * gc:48 * (gc + 1), :]
            for tt in range(16):
                rhs = srcs[:, 3 * tt:3 * (tt + 1), :].rearrange(
                    "p a b -> p (a b)")
                nc.tensor.matmul(vpx[:, :], msk[:, :], rhs,
                                 start=(gc == 0 and tt == 0),
                                 stop=(gc == 5 and tt == 15))

        # ---- iter 0: v0 from raw WU ----
        vpx0 = psR.tile([32, 480], f32, tag="vpx", name="vpx0")
        for gc in range(6):
            smult_mm_chunk(vpx0, gc, use_c=False)
        fold_squash(vpx0, 0.1)
        vrep_from_v(0.1)
        # ---- astep0 + softmax1 + smult1 pipelined per chunk ----
        vpx1 = psR.tile([32, 480], f32, tag="vpx", name="vpx1")
        for gc in range(6):
            astep_chunk(gc, first=True)
            softmax_chunk(gc)
            smult_mm_chunk(vpx1, gc, use_c=True)
        fold_squash(vpx1, 1.0)
        vrep_from_v(1.0)
        # ---- astep1 + softmax2 + smult2 pipelined per chunk ----
        vpx2 = psR.tile([32, 480], f32, tag="vpx", name="vpx2")
        for gc in range(6):
            astep_chunk(gc, first=False)
            softmax_chunk(gc)
            smult_mm_chunk(vpx2, gc, use_c=True)
        fold_squash(vpx2, 1.0)

        # ============== clf output: ||v_final|| = sqrt(sq) * fv ==============
        clf = smallp.tile([32, 10], f32)
        nc.scalar.activation(out=clf, in_=sq[:, :], func=AF.Sqrt,
                             bias=0.0, scale=1.0)
        nc.vector.tensor_tensor(out=clf, in0=clf, in1=fv, op=ALU.mult)
        nc.sync.dma_start(out=d["d_clf"][:, :], in_=clf)

        dbuf.release()
        routp.release()
        wup.release()
        psR.release()

        # ============== decoder ==============
        decp = tc.alloc_tile_pool(name="decp", bufs=1)
        psD = tc.alloc_tile_pool(name="psD", bufs=2, space="PSUM")
        oh = decp.tile([32, 10], f32)
        nc.sync.dma_start(out=oh, in_=d["d_oh"][:, :])
        ident = decp.tile([32, 32], bf16)
        nc.sync.dma_start(out=ident, in_=d["d_ident"][:, :])
        wd10 = decp.tile([128, 512], bf16)
        nc.sync.dma_start(out=wd10, in_=d["d_w1d0"][:, :])
        wd11 = decp.tile([32, 512], bf16)
        nc.sync.dma_start(out=wd11, in_=d["d_w1d1"][:, :])
        wd2 = decp.tile([128, 4, 1024], bf16)
        nc.sync.dma_start(out=wd2, in_=d["d_w2d"][:, :, :])
        wd3 = decp.tile([128, 8, 896], bf16)
        nc.sync.dma_start(out=wd3, in_=d["d_w3d"][:, :, :])
        bd1 = decp.tile([128, 4], f32)
        nc.sync.dma_start(out=bd1, in_=d["d_b1d"][:, :])
        bd2 = decp.tile([128, 8], f32)
        nc.sync.dma_start(out=bd2, in_=d["d_b2d"][:, :])
        bd3 = decp.tile([128, 7], f32)
        nc.sync.dma_start(out=bd3, in_=d["d_b3d"][:, :])

        mskd = decp.tile([32, 160], bf16)
        ohb = bass.AP(tensor=oh.tensor, offset=oh.offset,
                      ap=[oh.ap[0], [0, 16], [1, 10]])
        nc.vector.tensor_tensor(
            out=mskd[:, :].rearrange("p (e c) -> p e c", e=16),
            in0=v[:, :].rearrange("p (e c) -> p e c", e=16),
            in1=ohb, op=ALU.mult)
        # transpose masked v -> h_T chunks [128, 32] + [32, 32]
        ph0 = psD.tile([128, 32], bf16, tag="ph0")
        nc.tensor.transpose(ph0[:, :], mskd[:, 0:128], ident[:, :])
        ph1 = psD.tile([32, 32], bf16, tag="ph1")
        nc.tensor.transpose(ph1[:, :], mskd[:, 128:160], ident[:, :])
        h0 = decp.tile([128, 32], bf16)
        nc.vector.tensor_copy(out=h0, in_=ph0[:, :])
        h1 = decp.tile([32, 32], bf16)
        nc.vector.tensor_copy(out=h1, in_=ph1[:, :])

        hd1 = decp.tile([128, 4, 32], bf16)
        for mt in range(4):
            pd = psD.tile([128, 32], f32, tag="pd")
            nc.tensor.matmul(pd[:, :], wd10[:, 128 * mt:128 * (mt + 1)],
                             h0[:, :], start=True, stop=False)
            nc.tensor.matmul(pd[:, :], wd11[:, 128 * mt:128 * (mt + 1)],
                             h1[:, :], start=False, stop=True)
            nc.scalar.activation(out=hd1[:, mt, :], in_=pd[:, :], func=AF.Relu,
                                 bias=bd1[:, mt:mt + 1], scale=1.0)
        hd2 = decp.tile([128, 8, 32], bf16)
        for mt in range(8):
            pd = psD.tile([128, 32], f32, tag="pd")
            for kc in range(4):
                nc.tensor.matmul(pd[:, :], wd2[:, kc, 128 * mt:128 * (mt + 1)],
                                 hd1[:, kc, :], start=(kc == 0), stop=(kc == 3))
            nc.scalar.activation(out=hd2[:, mt, :], in_=pd[:, :], func=AF.Relu,
                                 bias=bd2[:, mt:mt + 1], scale=1.0)
        rec = decp.tile([128, 7, 32], f32)
        for mt in range(7):
            pd = psD.tile([128, 32], f32, tag="pd")
            for kc in range(8):
                nc.tensor.matmul(pd[:, :], wd3[:, kc, 128 * mt:128 * (mt + 1)],
                                 hd2[:, kc, :], start=(kc == 0), stop=(kc == 7))
            nc.scalar.activation(out=rec[:, mt, :], in_=pd[:, :], func=AF.Sigmoid,
                                 bias=bd3[:, mt:mt + 1], scale=1.0)
        nc.sync.dma_start(out=d["d_rec"][:, :, :], in_=rec)
        decp.release()
        psD.release()


def _prep_shared(conv1_w, conv1_b, pc_w, pc_b, routing_weights,
                 dec_w1, dec_b1, dec_w2, dec_b2, dec_w3, dec_b3):
    s = {}
    s["w1t"] = np.ascontiguousarray(
        conv1_w.transpose(2, 3, 1, 0).reshape(81, 256)).astype(npbf)
    s["b1c"] = np.ascontiguousarray(conv1_b.reshape(2, 128).T).astype(np.float32)
    s["pcwt"] = np.ascontiguousarray(
        pc_w.transpose(2, 3, 1, 0).reshape(81, 256, 256)).astype(npbf)
    s["pcbc"] = np.ascontiguousarray(pc_b.reshape(2, 128).T).astype(np.float32)
    # rw4[j, p, g=(h,yx), cd]: RW[n, cls, d, p], n = (8j+h)*36+yx
    R = routing_weights.reshape(4, 8, 36, 10, 16, 8)  # [j, h, yx, cls, d, p]
    s["rw4"] = np.ascontiguousarray(
        R.transpose(0, 5, 1, 2, 4, 3).reshape(4, 8, 288, 160)).astype(npbf)
    s["mask32"] = ((np.arange(128)[:, None] % 32) ==
                   np.arange(32)[None, :]).astype(npbf)
    s["ident"] = np.eye(32).astype(npbf)
    w1r = dec_w1.reshape(10, 16, 512).transpose(1, 0, 2).reshape(160, 512)
    s["w1d0"] = np.ascontiguousarray(w1r[:128]).astype(npbf)
    s["w1d1"] = np.ascontiguousarray(w1r[128:]).astype(npbf)
    s["w2d"] = np.ascontiguousarray(
        dec_w2.reshape(4, 128, 1024).transpose(1, 0, 2)).astype(npbf)
    w3p = np.concatenate([dec_w3, np.zeros((1024, 112), dec_w3.dtype)], axis=1)
    s["w3d"] = np.ascontiguousarray(
        w3p.reshape(8, 128, 896).transpose(1, 0, 2)).astype(npbf)
    s["b1d"] = np.ascontiguousarray(dec_b1.reshape(4, 128).T).astype(np.float32)
    s["b2d"] = np.ascontiguousarray(dec_b2.reshape(8, 128).T).astype(np.float32)
    b3p = np.concatenate([dec_b3, np.zeros(112, dec_b3.dtype)])
    s["b3d"] = np.ascontiguousarray(b3p.reshape(7, 128).T).astype(np.float32)
    return s


def _prep_core(inputs_sh, labels_sh):
    m = {}
    arr = np.asarray(inputs_sh[:, 0], np.float32)          # [32, 28, 28]
    A = np.empty((9, 9, 20, 20, 32), np.float32)
    for ky in range(9):
        for kx in range(9):
            A[ky, kx] = arr[:, ky:ky + 20, kx:kx + 20].transpose(1, 2, 0)
    m["imc"] = A.reshape(81, 400 * 32).astype(npbf)
    oh = np.zeros((32, 10), np.float32)
    oh[np.arange(32), np.asarray(labels_sh)] = 1.0
    m["onehot"] = oh
    return m


def kernel(inputs, labels, conv1_w, conv1_b, pc_w, pc_b, routing_weights,
           dec_w1, dec_b1, dec_w2, dec_b2, dec_w3, dec_b3):
    from concourse.bass_utils import run_bass_kernel_spmd
    if "nc" not in _CACHE:
        _CACHE["nc"] = _build()
    nc = _CACHE["nc"]

    shared = _prep_shared(np.asarray(conv1_w, np.float32),
                          np.asarray(conv1_b, np.float32),
                          np.asarray(pc_w, np.float32),
                          np.asarray(pc_b, np.float32),
                          np.asarray(routing_weights, np.float32),
                          np.asarray(dec_w1, np.float32),
                          np.asarray(dec_b1, np.float32),
                          np.asarray(dec_w2, np.float32),
                          np.asarray(dec_b2, np.float32),
                          np.asarray(dec_w3, np.float32),
                          np.asarray(dec_b3, np.float32))
    in_maps = []
    for i in range(8):
        sh = slice(32 * i, 32 * (i + 1))
        m = dict(shared)
        m.update(_prep_core(np.asarray(inputs, np.float32)[sh],
                            np.asarray(labels)[sh]))
        in_maps.append(m)

    res = run_bass_kernel_spmd(nc, in_maps, core_ids=list(range(8)))

    clf = np.concatenate([res.results[i]["clf_d"] for i in range(8)], axis=0)
    recs = []
    for i in range(8):
        rt = res.results[i]["recT_d"]            # [128, 7, 32]
        r = rt.transpose(1, 0, 2).reshape(896, 32)[:784].T   # [32, 784]
        recs.append(r.reshape(32, 1, 28, 28))
    rec = np.concatenate(recs, axis=0)
    return clf.astype(np.float32), rec.astype(np.float32)
